# revision 2
# baseline (speedup 1.0000x reference)
"""Trainium2 Bass kernel for nn_CoconAttention (dense transformer attention block).

Sharding: 8 cores = 4 batches x 2 head-groups (8 heads each). Each core gets
pre-transposed/sliced bf16 inputs (host pre-arranges every tensor into its
exact on-chip layout so all DMAs are contiguous), computes its partial output
outT [1024, 896] (bf16, transposed, pre-b_proj), and the host sums head-group
pairs + transposes.

Per core (H=8 heads, Dh=64, T=896, Tc=128, S=1024), bf16 compute / fp32 PSUM:
  qT/kT      : feature-major head-pair tiles (2 heads x 64 rows), split per
               token chunk (qT0/qT1) and ctx|t0 / t1 (kTa/kTb)
  scores^T   : [128 keys, 2 heads, tok] psum; exp on ACT -> bf16 probs
  probs^T    : masked via precomputed band masks (DVE mult), summed into dsum
  PV         : col-tiled matmuls, head hi -> psum partitions 64*hi..64*hi+64
  denom      : dsum (DVE bf16 accum over chunks) then ones[128,64]-stationary
               matmul -> denominator replicated across 64 partitions per head
  aT         : normalized via DVE reciprocal+mult, bf16
  out-proj   : per token-chunk, interleaved with the other chunk's attention
"""
import os
import sys

import numpy as np
import ml_dtypes

try:
    import concourse.bass as bass
except ImportError:  # fresh grading dir: fall back to the repo location
    sys.path.insert(0, "/opt/trn_rl_repo")
    import concourse.bass as bass
import concourse.bacc as bacc

import concourse.tile as tile
from concourse import mybir
from concourse.bass_utils import run_bass_kernel_spmd
from contextlib import ExitStack

F32 = mybir.dt.float32
BF16 = mybir.dt.bfloat16
AF = mybir.ActivationFunctionType

T, Tc, NX = 896, 128, 1024
TCH = ((0, 512), (512, 896))  # tok chunks
NPAIR = 4  # head pairs per core


def _rect(c, ts, te):
    """Live (unmasked) column range of scores chunk c within tok range [ts,te)."""
    cs = max(max(0, 128 * (c - 1)), ts)
    return None if cs >= te else (cs, te)


def _band_pieces(c, ts, te):
    """Mask applications for chunk c in [ts,te): (s0, e0, mask_col_offset)."""
    if c == 0:
        bs, be, moff, borig = 0, 128, 128, 0  # diag half only
    elif c <= 6:
        bs = 128 * (c - 1)
        be, moff, borig = bs + 256, 0, bs  # causal(128) + diag(128)
    else:
        bs, be, moff, borig = 768, 896, 0, 768  # causal half only
    s0, e0 = max(bs, ts), min(be, te)
    if s0 >= e0:
        return []
    return [(s0, e0, moff + (s0 - borig))]


def build_nc():
    nc = bacc.Bacc("TRN2", target_bir_lowering=False)

    # host pre-arranged layouts (partition-major, fully contiguous loads)
    x0_h = nc.dram_tensor("x0r", [128, 8, 512], BF16, kind="ExternalInput")
    x1_h = nc.dram_tensor("x1r", [128, 8, 384], BF16, kind="ExternalInput")
    ctx_h = nc.dram_tensor("ctxr", [128, 8, Tc], BF16, kind="ExternalInput")
    wq_h = nc.dram_tensor("w_q", [128, 4, 8, 128], BF16, kind="ExternalInput")
    wk_h = nc.dram_tensor("w_k", [128, 4, 8, 128], BF16, kind="ExternalInput")
    wv_h = nc.dram_tensor("w_v", [128, 8, 512], BF16, kind="ExternalInput")
    wkc_h = nc.dram_tensor("w_kc", [128, 4, 8, 128], BF16, kind="ExternalInput")
    wvc_h = nc.dram_tensor("w_vc", [128, 8, 512], BF16, kind="ExternalInput")
    wpj_h = nc.dram_tensor("w_pj", [128, 4, 1024], BF16, kind="ExternalInput")
    bqk_h = nc.dram_tensor("b_qk", [128, 8], F32, kind="ExternalInput")
    bkc_h = nc.dram_tensor("b_kc", [128, 4], F32, kind="ExternalInput")
    bv_h = nc.dram_tensor("b_v", [128, 512], BF16, kind="ExternalInput")
    bvc_h = nc.dram_tensor("b_vc", [128, 512], BF16, kind="ExternalInput")
    mb_h = nc.dram_tensor("mband", [128, 256], BF16, kind="ExternalInput")
    out_h = nc.dram_tensor("outT", [NX, T], BF16, kind="ExternalOutput")

    with tile.TileContext(nc) as tc, ExitStack() as top:
        consts = top.enter_context(tc.tile_pool(name="consts", bufs=1))
        wts = top.enter_context(tc.tile_pool(name="wts", bufs=1))
        xp = top.enter_context(tc.tile_pool(name="xp", bufs=1))
        qkp = top.enter_context(tc.tile_pool(name="qkp", bufs=1))
        vtp = top.enter_context(tc.tile_pool(name="vtp", bufs=1))
        atp = top.enter_context(tc.tile_pool(name="atp", bufs=1))
        probsp = top.enter_context(tc.tile_pool(name="probsp", bufs=4))
        dsp = top.enter_context(tc.tile_pool(name="dsp", bufs=2))
        rbp = top.enter_context(tc.tile_pool(name="rbp", bufs=2))
        outp = top.enter_context(tc.tile_pool(name="outp", bufs=3))
        # PSUM: pps 2x1 + scp 2x2 + pvp 2x1 = 8 banks
        pps = top.enter_context(tc.tile_pool(name="pps", bufs=2, space="PSUM"))
        scp = top.enter_context(tc.tile_pool(name="scp", bufs=2, space="PSUM"))
        pvp = top.enter_context(tc.tile_pool(name="pvp", bufs=2, space="PSUM"))

        # ---- constants ----
        ebias = consts.tile([128, 2], F32, name="ebias")  # exp bias: [0]=0, [1]=ctx -2
        nc.vector.memset(ebias[:, 0:1], 0.0)
        nc.vector.memset(ebias[:, 1:2], -2.0)
        ones64 = consts.tile([128, 64], BF16, name="ones64")
        nc.vector.memset(ones64, 1.0)
        maskband = consts.tile([128, 256], BF16, name="maskband")
        bias_qk = consts.tile([128, 8], F32, name="bias_qk")
        bias_kc = consts.tile([128, 4], F32, name="bias_kc")
        bvb = consts.tile([128, 512], BF16, name="bvb")
        bvcb = consts.tile([128, 512], BF16, name="bvcb")

        # ---- SBUF activation/weight tiles ----
        ctx_sb = wts.tile([128, 8, Tc], BF16, name="ctx_sb")
        wkc_sb = wts.tile([128, 4, 8, 128], BF16, name="wkc_sb")
        wvc_sb = wts.tile([128, 8, 512], BF16, name="wvc_sb")
        wq_sb = wts.tile([128, 4, 8, 128], BF16, name="wq_sb")
        wk_sb = wts.tile([128, 4, 8, 128], BF16, name="wk_sb")
        wv_sb = wts.tile([128, 8, 512], BF16, name="wv_sb")
        wpj_sb = wts.tile([128, 4, 1024], BF16, name="wpj_sb")
        x0_sb = xp.tile([128, 8, 512], BF16, name="x0_sb")
        x1_sb = xp.tile([128, 8, 384], BF16, name="x1_sb")

        # ---- input loads ----
        # sync HWDGE queue: critical-path order
        nc.sync.dma_start(out=ctx_sb, in_=ctx_h[:, :, :])
        nc.sync.dma_start(out=wkc_sb[:, 0, :, :], in_=wkc_h[:, 0, :, :])
        nc.sync.dma_start(out=wkc_sb[:, 1:4, :, :], in_=wkc_h[:, 1:4, :, :])
        nc.sync.dma_start(out=x0_sb[:, :, 0:256], in_=x0_h[:, :, 0:256])
        nc.sync.dma_start(out=x0_sb[:, :, 256:512], in_=x0_h[:, :, 256:512])
        for f in range(4):
            nc.sync.dma_start(out=wq_sb[:, f, :, :], in_=wq_h[:, f, :, :])
            nc.sync.dma_start(out=wk_sb[:, f, :, :], in_=wk_h[:, f, :, :])
        nc.sync.dma_start(out=x1_sb, in_=x1_h[:, :, :])
        nc.sync.dma_start(out=wpj_sb, in_=wpj_h[:, :, :])
        # scalar HWDGE queue: consts first (unblock ctx-proj drains), then bulk v
        nc.scalar.dma_start(out=bias_kc, in_=bkc_h[:, :])
        nc.scalar.dma_start(out=bias_qk, in_=bqk_h[:, :])
        nc.scalar.dma_start(out=maskband, in_=mb_h[:, :])
        nc.scalar.dma_start(out=bvcb, in_=bvc_h[:, :])
        nc.scalar.dma_start(out=bvb, in_=bv_h[:, :])
        nc.scalar.dma_start(out=wv_sb, in_=wv_h[:, :, :])
        nc.scalar.dma_start(out=wvc_sb, in_=wvc_h[:, :, :])

        # ---- persistent activation tiles (token-chunk-split: clean deps) ----
        qT0 = [qkp.tile([128, 512], BF16, name=f"qT0_{p}") for p in range(NPAIR)]
        qT1 = [qkp.tile([128, 384], BF16, name=f"qT1_{p}") for p in range(NPAIR)]
        kTa = [qkp.tile([128, 640], BF16, name=f"kTa{p}") for p in range(NPAIR)]
        kTb = [qkp.tile([128, 384], BF16, name=f"kTb{p}") for p in range(NPAIR)]
        v_sb = [vtp.tile([128, 8, 64], BF16, name=f"v{c}") for c in range(8)]
        aT0 = [atp.tile([128, 512], BF16, name=f"aT0_{p}") for p in range(NPAIR)]
        aT1 = [atp.tile([128, 384], BF16, name=f"aT1_{p}") for p in range(NPAIR)]

        def kt_slice(p, c):
            """kT columns [128c, 128c+128) of pair p (ctx + k concatenated)."""
            if c <= 4:
                return kTa[p][:, 128 * c:128 * c + 128]
            return kTb[p][:, 128 * c - 640:128 * c - 512]

        def x_slice(kc, ts, te):
            if te <= 512:
                return x0_sb[:, kc, ts:te]
            return x1_sb[:, kc, ts - 512:te - 512]

        # ---- ctx projections: kcT -> kTa cols 0:128, vc -> v_sb[0] ----
        for f in range(4):
            pt = pps.tile([128, 512], F32, tag="pp", name=f"pkc{f}")
            for kc in range(8):
                nc.tensor.matmul(
                    pt[:, 0:Tc], wkc_sb[:, f, kc, :],
                    ctx_sb[:, kc, :], start=(kc == 0), stop=(kc == 7))
            nc.scalar.activation(
                out=kTa[f][:, 0:Tc], in_=pt[:, 0:Tc], func=AF.Identity,
                bias=bias_kc[:, f:f + 1], scale=1.0)
        # ---- v projection (natural layout) ----
        def v_proj(tt):
            pt = pps.tile([128, 512], F32, tag="pp", name=f"pv{tt}")
            for kc in range(8):
                nc.tensor.matmul(
                    pt[:, 0:512], x_slice(kc, 128 * tt, 128 * tt + 128),
                    wv_sb[:, kc, :], start=(kc == 0), stop=(kc == 7))
            nc.vector.tensor_add(
                out=v_sb[1 + tt][:, :, :],
                in0=pt[:, 0:512].rearrange("p (h d) -> p h d", h=8),
                in1=bvb.rearrange("p (h d) -> p h d", h=8))

        for tt in range(4):
            v_proj(tt)

        pt = pps.tile([128, 512], F32, tag="pp", name="pvc")
        for kc in range(8):
            nc.tensor.matmul(
                pt[:, 0:512], ctx_sb[:, kc, :], wvc_sb[:, kc, :],
                start=(kc == 0), stop=(kc == 7))
        nc.vector.tensor_add(
            out=v_sb[0][:, :, :],
            in0=pt[:, 0:512].rearrange("p (h d) -> p h d", h=8),
            in1=bvcb.rearrange("p (h d) -> p h d", h=8))

        # ---- qT / kT projections (transposed layout), per token chunk ----
        def qk_ftile(w_sb, f, dest, dcol, bias_col, ts, te, drain):
            pt = pps.tile([128, 512], F32, tag="pp", name=f"pqk{bias_col}{ts}")
            for kc in range(8):
                nc.tensor.matmul(
                    pt[:, 0:te - ts], w_sb[:, f, kc, :],
                    x_slice(kc, ts, te), start=(kc == 0), stop=(kc == 7))
            if drain == "act":
                nc.scalar.activation(
                    out=dest[:, dcol:dcol + te - ts], in_=pt[:, 0:te - ts],
                    func=AF.Identity, bias=bias_qk[:, bias_col:bias_col + 1],
                    scale=1.0)
            else:
                nc.vector.tensor_scalar_add(
                    out=dest[:, dcol:dcol + te - ts], in0=pt[:, 0:te - ts],
                    scalar1=bias_qk[:, bias_col:bias_col + 1])

        def attn(p, t_i):
            ts, te = TCH[t_i]
            n = te - ts
            last_c = 4 if t_i == 0 else 7
            qT = qT0[p] if t_i == 0 else qT1[p]
            aT = aT0[p] if t_i == 0 else aT1[p]
            pa = pvp.tile([128, 512], F32, tag="pa", name=f"pa{p}{t_i}")
            dsum = dsp.tile([128, 2, 512], BF16, tag="ds", name=f"ds{p}{t_i}")
            chunks = [c for c in range(8) if _rect(c, ts, te) is not None]

            def scores(c):
                cs, _ = _rect(c, ts, te)
                sc = scp.tile([128, 2, 512], F32, tag="sc", name=f"sc{p}{t_i}{c}")
                for hi in range(2):
                    nc.tensor.matmul(
                        sc[:, hi, cs - ts:n],
                        kt_slice(p, c)[64 * hi:64 * hi + 64, :],
                        qT[64 * hi:64 * hi + 64, cs - ts:n],
                        start=True, stop=True, tile_position=(64 * hi, 0))
                pb = probsp.tile([128, 2, 512], BF16, tag="pb", name=f"pb{p}{t_i}{c}")
                nc.scalar.activation(
                    out=pb[:, :, cs - ts:n], in_=sc[:, :, cs - ts:n],
                    func=AF.Exp,
                    bias=(ebias[:, 1:2] if c == 0 else ebias[:, 0:1]),
                    scale=0.125)
                for hi in range(2):
                    for s0, e0, mc in _band_pieces(c, ts, te):
                        nc.vector.tensor_mul(
                            out=pb[:, hi, s0 - ts:e0 - ts],
                            in0=pb[:, hi, s0 - ts:e0 - ts],
                            in1=maskband[:, mc:mc + (e0 - s0)])
                return pb

            def pv(c, pb):
                cs, _ = _rect(c, ts, te)
                for hi in range(2):
                    nc.tensor.matmul(
                        pa[64 * hi:64 * hi + 64, cs - ts:n],
                        v_sb[c][:, 2 * p + hi, :],
                        pb[:, hi, cs - ts:n],
                        start=(c == 0), stop=(c == last_c),
                        skip_group_check=True, tile_position=(0, 64 * hi))
                if c == 0:
                    nc.vector.tensor_copy(out=dsum[:, :, 0:n], in_=pb[:, :, 0:n])
                else:
                    nc.vector.tensor_add(
                        out=dsum[:, :, cs - ts:n], in0=dsum[:, :, cs - ts:n],
                        in1=pb[:, :, cs - ts:n])

            pending = None
            for c in chunks:
                pb = scores(c)
                if pending is not None:
                    pv(*pending)
                pending = (c, pb)
            pv(*pending)
            pd = scp.tile([128, 2, 512], F32, tag="sc", name=f"pd{p}{t_i}")
            for hi in range(2):
                nc.tensor.matmul(
                    pd[64 * hi:64 * hi + 64, 0, 0:n], ones64, dsum[:, hi, 0:n],
                    start=True, stop=True, tile_position=(0, 64 * hi),
                    skip_group_check=True)
            rb = rbp.tile([128, 512], F32, tag="rb", name=f"rb{p}{t_i}")
            nc.vector.reciprocal(out=rb[:, 0:n], in_=pd[:, 0, 0:n])
            nc.vector.tensor_mul(out=aT[:, 0:n], in0=pa[:, 0:n], in1=rb[:, 0:n])

        def outproj(t_i, ofs):
            ts, te = TCH[t_i]
            n = te - ts
            aT = aT0 if t_i == 0 else aT1
            for of in ofs:
                pt = pps.tile([128, 512], F32, tag="pp", name=f"po{of}{t_i}")
                for kc in range(4):
                    nc.tensor.matmul(
                        pt[:, 0:n], wpj_sb[:, kc, 128 * of:128 * of + 128],
                        aT[kc][:, 0:n], start=(kc == 0), stop=(kc == 3))
                # t0 drains on DVE (ACT busy with t1 exps); t1 drains on ACT
                # (free after the last exp, DVE busy with denominators).
                # out DMAs alternate between the two HWDGE queues.
                ob = outp.tile([128, 512], BF16, tag="ob", name=f"ob{of}{t_i}")
                if t_i == 0:
                    nc.vector.tensor_copy(out=ob[:, 0:n], in_=pt[:, 0:n])
                else:
                    nc.scalar.copy(out=ob[:, 0:n], in_=pt[:, 0:n])
                eng = nc.sync if of % 2 == 0 else nc.scalar
                eng.dma_start(out=out_h[128 * of:128 * of + 128, ts:te], in_=ob[:, 0:n])

        def outproj_final(t_i):
            # of-pairs on scp-pool tiles (scores are done; 4 of-tiles in flight)
            ts, te = TCH[t_i]
            n = te - ts
            aT = aT0 if t_i == 0 else aT1
            for og in range(4):
                pt = scp.tile([128, 2, 512], F32, tag="sc", name=f"pg{og}{t_i}")
                for i in range(2):
                    of = 2 * og + i
                    for kc in range(4):
                        nc.tensor.matmul(
                            pt[:, i, 0:n], wpj_sb[:, kc, 128 * of:128 * of + 128],
                            aT[kc][:, 0:n], start=(kc == 0), stop=(kc == 3))
                ob = outp.tile([128, 2, 512], BF16, tag="obg", name=f"obg{og}{t_i}")
                nc.vector.tensor_copy(out=ob[0:64, :, 0:n], in_=pt[0:64, :, 0:n])
                nc.scalar.copy(out=ob[64:128, :, 0:n], in_=pt[64:128, :, 0:n])
                for i in range(2):
                    of = 2 * og + i
                    nc.sync.dma_start(
                        out=out_h[128 * of:128 * of + 64, ts:te], in_=ob[0:64, i, 0:n])
                    nc.scalar.dma_start(
                        out=out_h[128 * of + 64:128 * of + 128, ts:te],
                        in_=ob[64:128, i, 0:n])

        # ---- schedule: pair-major pipeline; outproj fills trailing stalls ----
        for p in range(NPAIR):
            qk_ftile(wq_sb, p, qT0[p], 0, p, 0, 512, "act")
            qk_ftile(wk_sb, p, kTa[p], Tc, 4 + p, 0, 512, "dve")
            attn(p, 0)
        for tt in range(4, 7):
            v_proj(tt)
        for p in range(NPAIR):
            qk_ftile(wq_sb, p, qT1[p], 0, p, 512, 896, "act")
            qk_ftile(wk_sb, p, kTb[p], 0, 4 + p, 512, 896, "dve")
            attn(p, 1)
            outproj(0, range(2 * p, 2 * p + 2))
        outproj(1, range(8))

    if not nc.is_finalized():
        nc.finalize()
    return nc


_NC_CACHE = {}


def _get_nc():
    if "nc" not in _NC_CACHE:
        _NC_CACHE["nc"] = build_nc()
    return _NC_CACHE["nc"]


def _pack128(v):
    """[128*n] -> [128, n] with [p, f] = v[128*f + p]."""
    n = v.shape[0] // 128
    return np.ascontiguousarray(v.reshape(n, 128).T)


def _kc_major(w):
    """[1024, F] -> [128, 8, F]: partition-major with kc chunks."""
    F = w.shape[1]
    return np.ascontiguousarray(w.reshape(8, 128, F).transpose(1, 0, 2))


def _f_major(w):
    """[1024, 512] -> [128, 4 fblk, 8 kc, 128]."""
    r = w.reshape(8, 128, 4, 128)  # kc, p, f, ff
    return np.ascontiguousarray(r.transpose(1, 2, 0, 3))


def make_in_maps(inputs):
    bf16 = ml_dtypes.bfloat16
    x = np.asarray(inputs["x"], np.float32)
    ctx_seq = np.asarray(inputs["context_seq"], np.float32)
    w_ref = np.asarray(inputs["w_ref"], np.float32)
    b_ref = np.asarray(inputs["b_ref"], np.float32)
    w_attn = np.asarray(inputs["w_attn"], np.float32)
    b_attn = np.asarray(inputs["b_attn"], np.float32)
    w_proj = np.asarray(inputs["w_proj"], np.float32)

    # mask band constant: cols 0-127 causal (1 where q>=p), cols 128-255
    # anti-diagonal (0 where q==p else 1)
    qq = np.arange(128)[None, :]
    pp = np.arange(128)[:, None]
    mband = np.ascontiguousarray(
        np.concatenate([(qq >= pp), (qq != pp)], axis=1).astype(bf16))

    in_maps = []
    for b in range(4):
        xT = x[b].T.astype(bf16)  # [1024, 896]
        x0r = _kc_major(xT[:, 0:512])
        x1r = _kc_major(xT[:, 512:896])
        ctxr = _kc_major(ctx_seq[b].T.astype(bf16))
        for g in range(2):
            sl = slice(512 * g, 512 * g + 512)
            in_maps.append(dict(
                x0r=x0r,
                x1r=x1r,
                ctxr=ctxr,
                w_q=_f_major(w_attn[:, 0 * NX:1 * NX][:, sl].astype(bf16)),
                w_k=_f_major(w_attn[:, 1 * NX:2 * NX][:, sl].astype(bf16)),
                w_v=_kc_major(w_attn[:, 2 * NX:3 * NX][:, sl].astype(bf16)),
                w_kc=_f_major(w_ref[:, 0 * NX:1 * NX][:, sl].astype(bf16)),
                w_vc=_kc_major(w_ref[:, 1 * NX:2 * NX][:, sl].astype(bf16)),
                w_pj=np.ascontiguousarray(
                    w_proj[sl, :].astype(bf16).reshape(4, 128, NX).transpose(1, 0, 2)),
                b_qk=_pack128(np.concatenate([b_attn[0 * NX:1 * NX][sl],
                                              b_attn[1 * NX:2 * NX][sl]])),
                b_kc=_pack128(b_ref[0 * NX:1 * NX][sl]),
                b_v=np.ascontiguousarray(np.broadcast_to(
                    b_attn[2 * NX:3 * NX][sl].astype(bf16), (128, 512))),
                b_vc=np.ascontiguousarray(np.broadcast_to(
                    b_ref[1 * NX:2 * NX][sl].astype(bf16), (128, 512))),
                mband=mband,
            ))
    return in_maps


def kernel(**inputs):
    b_proj = np.asarray(inputs["b_proj"], np.float32)
    in_maps = make_in_maps(inputs)
    nc = _get_nc()
    res = run_bass_kernel_spmd(nc, in_maps, core_ids=list(range(8)),
                               trace=os.environ.get("COCON_TRACE", "") == "1")
    outs = res.results
    out = np.empty((4, T, NX), np.float32)
    for b in range(4):
        acc = (outs[2 * b]["outT"].astype(np.float32)
               + outs[2 * b + 1]["outT"].astype(np.float32))  # [1024, 896]
        out[b] = acc.T + b_proj[None, :]
    if res.exec_time_ns is not None:
        kernel.last_exec_time_ns = res.exec_time_ns
    return out


kernel.last_exec_time_ns = None


# revision 3
# speedup vs baseline: 1.0846x; 1.0846x over previous
"""Trainium2 Bass kernel for nn_CoconAttention (dense transformer attention block).

Sharding: 8 cores = 4 batches x 2 head-groups (8 heads each). Each core gets
pre-transposed/sliced bf16 inputs (host pre-arranges every tensor into its
exact on-chip layout so all DMAs are contiguous), computes its partial output
outT [1024, 896] (bf16, transposed, pre-b_proj), and the host sums head-group
pairs + transposes.

Per core (H=8 heads, Dh=64, T=896, Tc=128, S=1024), bf16 compute / fp32 PSUM:
  qT/kT      : feature-major head-pair tiles (2 heads x 64 rows), split per
               token chunk (qT0/qT1) and ctx|t0 / t1 (kTa/kTb)
  scores^T   : [128 keys, 2 heads, tok] psum; exp on ACT -> bf16 probs
  probs^T    : masked via precomputed band masks (DVE mult), summed into dsum
  PV         : col-tiled matmuls, head hi -> psum partitions 64*hi..64*hi+64
  denom      : dsum (DVE bf16 accum over chunks) then ones[128,64]-stationary
               matmul -> denominator replicated across 64 partitions per head
  aT         : normalized via DVE reciprocal+mult, bf16
  out-proj   : per token-chunk, interleaved with the other chunk's attention
"""
import os
import sys

import numpy as np
import ml_dtypes

try:
    import concourse.bass as bass
except ImportError:  # fresh grading dir: fall back to the repo location
    sys.path.insert(0, "/opt/trn_rl_repo")
    import concourse.bass as bass
import concourse.bacc as bacc

import concourse.tile as tile
from concourse import mybir
from concourse.bass_utils import run_bass_kernel_spmd
from contextlib import ExitStack

F32 = mybir.dt.float32
BF16 = mybir.dt.bfloat16
AF = mybir.ActivationFunctionType

T, Tc, NX = 896, 128, 1024
TCH = ((0, 512), (512, 896))  # tok chunks
NPAIR = 4  # head pairs per core


def _rect(c, ts, te):
    """Live (unmasked) column range of scores chunk c within tok range [ts,te)."""
    cs = max(max(0, 128 * (c - 1)), ts)
    return None if cs >= te else (cs, te)


def _band_pieces(c, ts, te):
    """Mask applications for chunk c in [ts,te): (s0, e0, mask_col_offset)."""
    if c == 0:
        bs, be, moff, borig = 0, 128, 128, 0  # diag half only
    elif c <= 6:
        bs = 128 * (c - 1)
        be, moff, borig = bs + 256, 0, bs  # causal(128) + diag(128)
    else:
        bs, be, moff, borig = 768, 896, 0, 768  # causal half only
    s0, e0 = max(bs, ts), min(be, te)
    if s0 >= e0:
        return []
    return [(s0, e0, moff + (s0 - borig))]


def build_nc():
    nc = bacc.Bacc("TRN2", target_bir_lowering=False)

    # host pre-arranged layouts (partition-major, fully contiguous loads)
    x0_h = nc.dram_tensor("x0r", [128, 8, 512], BF16, kind="ExternalInput")
    x1_h = nc.dram_tensor("x1r", [128, 8, 384], BF16, kind="ExternalInput")
    ctx_h = nc.dram_tensor("ctxr", [128, 8, Tc], BF16, kind="ExternalInput")
    wq_h = nc.dram_tensor("w_q", [128, 4, 8, 128], BF16, kind="ExternalInput")
    wk_h = nc.dram_tensor("w_k", [128, 4, 8, 128], BF16, kind="ExternalInput")
    wv_h = nc.dram_tensor("w_v", [128, 8, 512], BF16, kind="ExternalInput")
    wkc_h = nc.dram_tensor("w_kc", [128, 4, 8, 128], BF16, kind="ExternalInput")
    wvc_h = nc.dram_tensor("w_vc", [128, 8, 512], BF16, kind="ExternalInput")
    wpj_h = nc.dram_tensor("w_pj", [128, 4, 1024], BF16, kind="ExternalInput")
    bqk_h = nc.dram_tensor("b_qk", [128, 8], F32, kind="ExternalInput")
    bkc_h = nc.dram_tensor("b_kc", [128, 4], F32, kind="ExternalInput")
    bv_h = nc.dram_tensor("b_v", [128, 512], BF16, kind="ExternalInput")
    bvc_h = nc.dram_tensor("b_vc", [128, 512], BF16, kind="ExternalInput")
    mb_h = nc.dram_tensor("mband", [128, 256], BF16, kind="ExternalInput")
    out_h = nc.dram_tensor("outT", [NX, T], BF16, kind="ExternalOutput")

    with tile.TileContext(nc) as tc, ExitStack() as top:
        consts = top.enter_context(tc.tile_pool(name="consts", bufs=1))
        wts = top.enter_context(tc.tile_pool(name="wts", bufs=1))
        xp = top.enter_context(tc.tile_pool(name="xp", bufs=1))
        qkp = top.enter_context(tc.tile_pool(name="qkp", bufs=1))
        vtp = top.enter_context(tc.tile_pool(name="vtp", bufs=1))
        atp = top.enter_context(tc.tile_pool(name="atp", bufs=1))
        probsp = top.enter_context(tc.tile_pool(name="probsp", bufs=4))
        dsp = top.enter_context(tc.tile_pool(name="dsp", bufs=2))
        rbp = top.enter_context(tc.tile_pool(name="rbp", bufs=2))
        outp = top.enter_context(tc.tile_pool(name="outp", bufs=3))
        # PSUM: pps 2x1 + scp 2x2 + pvp 2x1 = 8 banks
        pps = top.enter_context(tc.tile_pool(name="pps", bufs=2, space="PSUM"))
        scp = top.enter_context(tc.tile_pool(name="scp", bufs=2, space="PSUM"))
        pvp = top.enter_context(tc.tile_pool(name="pvp", bufs=2, space="PSUM"))

        # ---- constants ----
        ebias = consts.tile([128, 2], F32, name="ebias")  # exp bias: [0]=0, [1]=ctx -2
        nc.vector.memset(ebias[:, 0:1], 0.0)
        nc.vector.memset(ebias[:, 1:2], -2.0)
        ones64 = consts.tile([128, 64], BF16, name="ones64")
        nc.vector.memset(ones64, 1.0)
        maskband = consts.tile([128, 256], BF16, name="maskband")
        bias_qk = consts.tile([128, 8], F32, name="bias_qk")
        bias_kc = consts.tile([128, 4], F32, name="bias_kc")
        bvb = consts.tile([128, 512], BF16, name="bvb")
        bvcb = consts.tile([128, 512], BF16, name="bvcb")

        # ---- SBUF activation/weight tiles ----
        ctx_sb = wts.tile([128, 8, Tc], BF16, name="ctx_sb")
        wkc_sb = wts.tile([128, 4, 8, 128], BF16, name="wkc_sb")
        wvc_sb = wts.tile([128, 8, 512], BF16, name="wvc_sb")
        wq_sb = wts.tile([128, 4, 8, 128], BF16, name="wq_sb")
        wk_sb = wts.tile([128, 4, 8, 128], BF16, name="wk_sb")
        wv_sb = wts.tile([128, 8, 512], BF16, name="wv_sb")
        wpj_sb = wts.tile([128, 4, 1024], BF16, name="wpj_sb")
        x0_sb = xp.tile([128, 8, 512], BF16, name="x0_sb")
        x1_sb = xp.tile([128, 8, 384], BF16, name="x1_sb")

        # ---- input loads ----
        # sync HWDGE queue: critical-path order
        nc.sync.dma_start(out=ctx_sb, in_=ctx_h[:, :, :])
        nc.sync.dma_start(out=wkc_sb[:, 0, :, :], in_=wkc_h[:, 0, :, :])
        nc.sync.dma_start(out=x0_sb[:, :, 0:256], in_=x0_h[:, :, 0:256])
        nc.sync.dma_start(out=wkc_sb[:, 1:4, :, :], in_=wkc_h[:, 1:4, :, :])
        nc.sync.dma_start(out=x0_sb[:, :, 256:512], in_=x0_h[:, :, 256:512])
        for f in range(4):
            nc.sync.dma_start(out=wq_sb[:, f, :, :], in_=wq_h[:, f, :, :])
            nc.sync.dma_start(out=wk_sb[:, f, :, :], in_=wk_h[:, f, :, :])
        nc.sync.dma_start(out=x1_sb, in_=x1_h[:, :, :])
        nc.sync.dma_start(out=wpj_sb, in_=wpj_h[:, :, :])
        # scalar HWDGE queue: consts first (unblock ctx-proj drains), then bulk v
        nc.scalar.dma_start(out=bias_kc, in_=bkc_h[:, :])
        nc.scalar.dma_start(out=bias_qk, in_=bqk_h[:, :])
        nc.scalar.dma_start(out=maskband, in_=mb_h[:, :])
        nc.scalar.dma_start(out=bvcb, in_=bvc_h[:, :])
        nc.scalar.dma_start(out=bvb, in_=bv_h[:, :])
        nc.scalar.dma_start(out=wv_sb, in_=wv_h[:, :, :])
        nc.scalar.dma_start(out=wvc_sb, in_=wvc_h[:, :, :])

        # ---- persistent activation tiles (token-chunk-split: clean deps) ----
        qT0 = [qkp.tile([128, 512], BF16, name=f"qT0_{p}") for p in range(NPAIR)]
        qT1 = [qkp.tile([128, 384], BF16, name=f"qT1_{p}") for p in range(NPAIR)]
        kTa = [qkp.tile([128, 640], BF16, name=f"kTa{p}") for p in range(NPAIR)]
        kTb = [qkp.tile([128, 384], BF16, name=f"kTb{p}") for p in range(NPAIR)]
        v_sb = [vtp.tile([128, 8, 64], BF16, name=f"v{c}") for c in range(8)]
        aT0 = [atp.tile([128, 512], BF16, name=f"aT0_{p}") for p in range(NPAIR)]
        aT1 = [atp.tile([128, 384], BF16, name=f"aT1_{p}") for p in range(NPAIR)]

        def kt_slice(p, c):
            """kT columns [128c, 128c+128) of pair p (ctx + k concatenated)."""
            if c <= 4:
                return kTa[p][:, 128 * c:128 * c + 128]
            return kTb[p][:, 128 * c - 640:128 * c - 512]

        def x_slice(kc, ts, te):
            if te <= 512:
                return x0_sb[:, kc, ts:te]
            return x1_sb[:, kc, ts - 512:te - 512]

        # ---- ctx projections: kcT -> kTa cols 0:128, vc -> v_sb[0] ----
        for f in range(4):
            pt = pps.tile([128, 512], F32, tag="pp", name=f"pkc{f}")
            for kc in range(8):
                nc.tensor.matmul(
                    pt[:, 0:Tc], wkc_sb[:, f, kc, :],
                    ctx_sb[:, kc, :], start=(kc == 0), stop=(kc == 7))
            nc.scalar.activation(
                out=kTa[f][:, 0:Tc], in_=pt[:, 0:Tc], func=AF.Identity,
                bias=bias_kc[:, f:f + 1], scale=1.0)
        # ---- v projection (natural layout) ----
        def v_proj(tt):
            pt = pps.tile([128, 512], F32, tag="pp", name=f"pv{tt}")
            for kc in range(8):
                nc.tensor.matmul(
                    pt[:, 0:512], x_slice(kc, 128 * tt, 128 * tt + 128),
                    wv_sb[:, kc, :], start=(kc == 0), stop=(kc == 7))
            nc.vector.tensor_add(
                out=v_sb[1 + tt][:, :, :],
                in0=pt[:, 0:512].rearrange("p (h d) -> p h d", h=8),
                in1=bvb.rearrange("p (h d) -> p h d", h=8))

        for tt in range(4):
            v_proj(tt)

        pt = pps.tile([128, 512], F32, tag="pp", name="pvc")
        for kc in range(8):
            nc.tensor.matmul(
                pt[:, 0:512], ctx_sb[:, kc, :], wvc_sb[:, kc, :],
                start=(kc == 0), stop=(kc == 7))
        nc.vector.tensor_add(
            out=v_sb[0][:, :, :],
            in0=pt[:, 0:512].rearrange("p (h d) -> p h d", h=8),
            in1=bvcb.rearrange("p (h d) -> p h d", h=8))

        # ---- qT / kT projections (transposed layout), per token chunk ----
        def qk_ftile(w_sb, f, dest, dcol, bias_col, ts, te, drain):
            pt = pps.tile([128, 512], F32, tag="pp", name=f"pqk{bias_col}{ts}")
            for kc in range(8):
                nc.tensor.matmul(
                    pt[:, 0:te - ts], w_sb[:, f, kc, :],
                    x_slice(kc, ts, te), start=(kc == 0), stop=(kc == 7))
            if drain == "act":
                nc.scalar.activation(
                    out=dest[:, dcol:dcol + te - ts], in_=pt[:, 0:te - ts],
                    func=AF.Identity, bias=bias_qk[:, bias_col:bias_col + 1],
                    scale=1.0)
            else:
                nc.vector.tensor_scalar_add(
                    out=dest[:, dcol:dcol + te - ts], in0=pt[:, 0:te - ts],
                    scalar1=bias_qk[:, bias_col:bias_col + 1])

        def attn(p, t_i):
            ts, te = TCH[t_i]
            n = te - ts
            last_c = 4 if t_i == 0 else 7
            qT = qT0[p] if t_i == 0 else qT1[p]
            aT = aT0[p] if t_i == 0 else aT1[p]
            pa = pvp.tile([128, 512], F32, tag="pa", name=f"pa{p}{t_i}")
            dsum = dsp.tile([128, 2, 512], BF16, tag="ds", name=f"ds{p}{t_i}")
            chunks = [c for c in range(8) if _rect(c, ts, te) is not None]

            def scores(c):
                cs, _ = _rect(c, ts, te)
                sc = scp.tile([128, 2, 512], F32, tag="sc", name=f"sc{p}{t_i}{c}")
                for hi in range(2):
                    nc.tensor.matmul(
                        sc[:, hi, cs - ts:n],
                        kt_slice(p, c)[64 * hi:64 * hi + 64, :],
                        qT[64 * hi:64 * hi + 64, cs - ts:n],
                        start=True, stop=True, tile_position=(64 * hi, 0))
                pb = probsp.tile([128, 2, 512], BF16, tag="pb", name=f"pb{p}{t_i}{c}")
                nc.scalar.activation(
                    out=pb[:, :, cs - ts:n], in_=sc[:, :, cs - ts:n],
                    func=AF.Exp,
                    bias=(ebias[:, 1:2] if c == 0 else ebias[:, 0:1]),
                    scale=0.125)
                for hi in range(2):
                    for s0, e0, mc in _band_pieces(c, ts, te):
                        nc.vector.tensor_mul(
                            out=pb[:, hi, s0 - ts:e0 - ts],
                            in0=pb[:, hi, s0 - ts:e0 - ts],
                            in1=maskband[:, mc:mc + (e0 - s0)])
                return pb

            def pv(c, pb):
                cs, _ = _rect(c, ts, te)
                for hi in range(2):
                    nc.tensor.matmul(
                        pa[64 * hi:64 * hi + 64, cs - ts:n],
                        v_sb[c][:, 2 * p + hi, :],
                        pb[:, hi, cs - ts:n],
                        start=(c == 0), stop=(c == last_c),
                        skip_group_check=True, tile_position=(0, 64 * hi))
                if c == 0:
                    nc.vector.tensor_copy(out=dsum[:, :, 0:n], in_=pb[:, :, 0:n])
                else:
                    nc.vector.tensor_add(
                        out=dsum[:, :, cs - ts:n], in0=dsum[:, :, cs - ts:n],
                        in1=pb[:, :, cs - ts:n])

            pending = None
            for c in chunks:
                pb = scores(c)
                if pending is not None:
                    pv(*pending)
                pending = (c, pb)
            pv(*pending)
            pd = scp.tile([128, 2, 512], F32, tag="sc", name=f"pd{p}{t_i}")
            for hi in range(2):
                nc.tensor.matmul(
                    pd[64 * hi:64 * hi + 64, 0, 0:n], ones64, dsum[:, hi, 0:n],
                    start=True, stop=True, tile_position=(0, 64 * hi),
                    skip_group_check=True)
            rb = rbp.tile([128, 512], F32, tag="rb", name=f"rb{p}{t_i}")
            nc.vector.reciprocal(out=rb[:, 0:n], in_=pd[:, 0, 0:n])
            nc.vector.tensor_mul(out=aT[:, 0:n], in0=pa[:, 0:n], in1=rb[:, 0:n])

        def outproj(t_i, ofs):
            ts, te = TCH[t_i]
            n = te - ts
            aT = aT0 if t_i == 0 else aT1
            for of in ofs:
                pt = pps.tile([128, 512], F32, tag="pp", name=f"po{of}{t_i}")
                for kc in range(4):
                    nc.tensor.matmul(
                        pt[:, 0:n], wpj_sb[:, kc, 128 * of:128 * of + 128],
                        aT[kc][:, 0:n], start=(kc == 0), stop=(kc == 3))
                # t0 drains on DVE (ACT busy with t1 exps); t1 drains on ACT
                # (free after the last exp, DVE busy with denominators).
                # out DMAs alternate between the two HWDGE queues.
                ob = outp.tile([128, 512], BF16, tag="ob", name=f"ob{of}{t_i}")
                if t_i == 0:
                    nc.vector.tensor_copy(out=ob[:, 0:n], in_=pt[:, 0:n])
                else:
                    nc.scalar.copy(out=ob[:, 0:n], in_=pt[:, 0:n])
                eng = nc.sync if of % 2 == 0 else nc.scalar
                eng.dma_start(out=out_h[128 * of:128 * of + 128, ts:te], in_=ob[:, 0:n])

        def outproj_final(t_i):
            # of-pairs on scp-pool tiles (scores are done; 4 of-tiles in flight)
            ts, te = TCH[t_i]
            n = te - ts
            aT = aT0 if t_i == 0 else aT1
            for og in range(4):
                pt = scp.tile([128, 2, 512], F32, tag="sc", name=f"pg{og}{t_i}")
                for i in range(2):
                    of = 2 * og + i
                    for kc in range(4):
                        nc.tensor.matmul(
                            pt[:, i, 0:n], wpj_sb[:, kc, 128 * of:128 * of + 128],
                            aT[kc][:, 0:n], start=(kc == 0), stop=(kc == 3))
                ob = outp.tile([128, 2, 512], BF16, tag="obg", name=f"obg{og}{t_i}")
                nc.vector.tensor_copy(out=ob[0:64, :, 0:n], in_=pt[0:64, :, 0:n])
                nc.scalar.copy(out=ob[64:128, :, 0:n], in_=pt[64:128, :, 0:n])
                for i in range(2):
                    of = 2 * og + i
                    nc.sync.dma_start(
                        out=out_h[128 * of:128 * of + 64, ts:te], in_=ob[0:64, i, 0:n])
                    nc.scalar.dma_start(
                        out=out_h[128 * of + 64:128 * of + 128, ts:te],
                        in_=ob[64:128, i, 0:n])

        # ---- schedule: pair-major pipeline; outproj fills trailing stalls ----
        for p in range(NPAIR):
            qk_ftile(wq_sb, p, qT0[p], 0, p, 0, 512, "act")
            qk_ftile(wk_sb, p, kTa[p], Tc, 4 + p, 0, 512, "dve")
            attn(p, 0)
        for tt in range(4, 7):
            v_proj(tt)
        for p in range(NPAIR):
            qk_ftile(wq_sb, p, qT1[p], 0, p, 512, 896, "act")
            qk_ftile(wk_sb, p, kTb[p], 0, 4 + p, 512, 896, "dve")
            attn(p, 1)
            outproj(0, range(2 * p, 2 * p + 2))
        outproj(1, range(8))

    if not nc.is_finalized():
        nc.finalize()
    return nc


_NC_CACHE = {}


def _get_nc():
    if "nc" not in _NC_CACHE:
        _NC_CACHE["nc"] = build_nc()
    return _NC_CACHE["nc"]


def _pack128(v):
    """[128*n] -> [128, n] with [p, f] = v[128*f + p]."""
    n = v.shape[0] // 128
    return np.ascontiguousarray(v.reshape(n, 128).T)


def _kc_major(w):
    """[1024, F] -> [128, 8, F]: partition-major with kc chunks."""
    F = w.shape[1]
    return np.ascontiguousarray(w.reshape(8, 128, F).transpose(1, 0, 2))


def _f_major(w):
    """[1024, 512] -> [128, 4 fblk, 8 kc, 128]."""
    r = w.reshape(8, 128, 4, 128)  # kc, p, f, ff
    return np.ascontiguousarray(r.transpose(1, 2, 0, 3))


def make_in_maps(inputs):
    bf16 = ml_dtypes.bfloat16
    x = np.asarray(inputs["x"], np.float32)
    ctx_seq = np.asarray(inputs["context_seq"], np.float32)
    w_ref = np.asarray(inputs["w_ref"], np.float32)
    b_ref = np.asarray(inputs["b_ref"], np.float32)
    w_attn = np.asarray(inputs["w_attn"], np.float32)
    b_attn = np.asarray(inputs["b_attn"], np.float32)
    w_proj = np.asarray(inputs["w_proj"], np.float32)

    # mask band constant: cols 0-127 causal (1 where q>=p), cols 128-255
    # anti-diagonal (0 where q==p else 1)
    qq = np.arange(128)[None, :]
    pp = np.arange(128)[:, None]
    mband = np.ascontiguousarray(
        np.concatenate([(qq >= pp), (qq != pp)], axis=1).astype(bf16))

    in_maps = []
    for b in range(4):
        xT = x[b].T.astype(bf16)  # [1024, 896]
        x0r = _kc_major(xT[:, 0:512])
        x1r = _kc_major(xT[:, 512:896])
        ctxr = _kc_major(ctx_seq[b].T.astype(bf16))
        for g in range(2):
            sl = slice(512 * g, 512 * g + 512)
            in_maps.append(dict(
                x0r=x0r,
                x1r=x1r,
                ctxr=ctxr,
                w_q=_f_major(w_attn[:, 0 * NX:1 * NX][:, sl].astype(bf16)),
                w_k=_f_major(w_attn[:, 1 * NX:2 * NX][:, sl].astype(bf16)),
                w_v=_kc_major(w_attn[:, 2 * NX:3 * NX][:, sl].astype(bf16)),
                w_kc=_f_major(w_ref[:, 0 * NX:1 * NX][:, sl].astype(bf16)),
                w_vc=_kc_major(w_ref[:, 1 * NX:2 * NX][:, sl].astype(bf16)),
                w_pj=np.ascontiguousarray(
                    w_proj[sl, :].astype(bf16).reshape(4, 128, NX).transpose(1, 0, 2)),
                b_qk=_pack128(np.concatenate([b_attn[0 * NX:1 * NX][sl],
                                              b_attn[1 * NX:2 * NX][sl]])),
                b_kc=_pack128(b_ref[0 * NX:1 * NX][sl]),
                b_v=np.ascontiguousarray(np.broadcast_to(
                    b_attn[2 * NX:3 * NX][sl].astype(bf16), (128, 512))),
                b_vc=np.ascontiguousarray(np.broadcast_to(
                    b_ref[1 * NX:2 * NX][sl].astype(bf16), (128, 512))),
                mband=mband,
            ))
    return in_maps


def kernel(**inputs):
    b_proj = np.asarray(inputs["b_proj"], np.float32)
    in_maps = make_in_maps(inputs)
    nc = _get_nc()
    res = run_bass_kernel_spmd(nc, in_maps, core_ids=list(range(8)),
                               trace=os.environ.get("COCON_TRACE", "") == "1")
    outs = res.results
    out = np.empty((4, T, NX), np.float32)
    for b in range(4):
        acc = (outs[2 * b]["outT"].astype(np.float32)
               + outs[2 * b + 1]["outT"].astype(np.float32))  # [1024, 896]
        out[b] = acc.T + b_proj[None, :]
    if res.exec_time_ns is not None:
        kernel.last_exec_time_ns = res.exec_time_ns
    return out


kernel.last_exec_time_ns = None


# revision 4
# speedup vs baseline: 1.0938x; 1.0085x over previous
"""Trainium2 Bass kernel for nn_CoconAttention (dense transformer attention block).

Sharding: 8 cores = 4 batches x 2 head-groups (8 heads each). Each core gets
pre-transposed/sliced bf16 inputs (host pre-arranges every tensor into its
exact on-chip layout so all DMAs are contiguous), computes its partial output
outT [1024, 896] (bf16, transposed, pre-b_proj), and the host sums head-group
pairs + transposes.

Per core (H=8 heads, Dh=64, T=896, Tc=128, S=1024), bf16 compute / fp32 PSUM:
  qT/kT      : feature-major head-pair tiles (2 heads x 64 rows), split per
               token chunk (qT0/qT1) and ctx|t0 / t1 (kTa/kTb)
  scores^T   : [128 keys, 2 heads, tok] psum; exp on ACT -> bf16 probs
  probs^T    : masked via precomputed band masks (DVE mult), summed into dsum
  PV         : col-tiled matmuls, head hi -> psum partitions 64*hi..64*hi+64
  denom      : dsum (DVE bf16 accum over chunks) then ones[128,64]-stationary
               matmul -> denominator replicated across 64 partitions per head
  aT         : normalized via DVE reciprocal+mult, bf16
  out-proj   : per token-chunk, interleaved with the other chunk's attention
"""
import os
import sys

import numpy as np
import ml_dtypes

try:
    import concourse.bass as bass
except ImportError:  # fresh grading dir: fall back to the repo location
    sys.path.insert(0, "/opt/trn_rl_repo")
    import concourse.bass as bass
import concourse.bacc as bacc

import concourse.tile as tile
from concourse import mybir
from concourse.bass_utils import run_bass_kernel_spmd
from contextlib import ExitStack

F32 = mybir.dt.float32
BF16 = mybir.dt.bfloat16
AF = mybir.ActivationFunctionType

T, Tc, NX = 896, 128, 1024
TCH = ((0, 512), (512, 896))  # tok chunks
NPAIR = 4  # head pairs per core


def _rect(c, ts, te):
    """Live (unmasked) column range of scores chunk c within tok range [ts,te)."""
    cs = max(max(0, 128 * (c - 1)), ts)
    return None if cs >= te else (cs, te)


def _band_pieces(c, ts, te):
    """Mask applications for chunk c in [ts,te): (s0, e0, mask_col_offset)."""
    if c == 0:
        bs, be, moff, borig = 0, 128, 128, 0  # diag half only
    elif c <= 6:
        bs = 128 * (c - 1)
        be, moff, borig = bs + 256, 0, bs  # causal(128) + diag(128)
    else:
        bs, be, moff, borig = 768, 896, 0, 768  # causal half only
    s0, e0 = max(bs, ts), min(be, te)
    if s0 >= e0:
        return []
    return [(s0, e0, moff + (s0 - borig))]


def build_nc():
    nc = bacc.Bacc("TRN2", target_bir_lowering=False)

    # host pre-arranged layouts (partition-major, fully contiguous loads)
    x0_h = nc.dram_tensor("x0r", [128, 8, 512], BF16, kind="ExternalInput")
    x1_h = nc.dram_tensor("x1r", [128, 8, 384], BF16, kind="ExternalInput")
    ctx_h = nc.dram_tensor("ctxr", [128, 8, Tc], BF16, kind="ExternalInput")
    wq_h = nc.dram_tensor("w_q", [128, 4, 8, 128], BF16, kind="ExternalInput")
    wk_h = nc.dram_tensor("w_k", [128, 4, 8, 128], BF16, kind="ExternalInput")
    wv_h = nc.dram_tensor("w_v", [128, 8, 512], BF16, kind="ExternalInput")
    wkc_h = nc.dram_tensor("w_kc", [128, 4, 8, 128], BF16, kind="ExternalInput")
    wvc_h = nc.dram_tensor("w_vc", [128, 8, 512], BF16, kind="ExternalInput")
    wpj_h = nc.dram_tensor("w_pj", [128, 4, 1024], BF16, kind="ExternalInput")
    bqk_h = nc.dram_tensor("b_qk", [128, 8], F32, kind="ExternalInput")
    bkc_h = nc.dram_tensor("b_kc", [128, 4], F32, kind="ExternalInput")
    bv_h = nc.dram_tensor("b_v", [128, 512], BF16, kind="ExternalInput")
    bvc_h = nc.dram_tensor("b_vc", [128, 512], BF16, kind="ExternalInput")
    mb_h = nc.dram_tensor("mband", [128, 256], BF16, kind="ExternalInput")
    out_h = nc.dram_tensor("outT", [NX, T], BF16, kind="ExternalOutput")

    with tile.TileContext(nc) as tc, ExitStack() as top:
        consts = top.enter_context(tc.tile_pool(name="consts", bufs=1))
        wts = top.enter_context(tc.tile_pool(name="wts", bufs=1))
        xp = top.enter_context(tc.tile_pool(name="xp", bufs=1))
        qkp = top.enter_context(tc.tile_pool(name="qkp", bufs=1))
        vtp = top.enter_context(tc.tile_pool(name="vtp", bufs=1))
        atp = top.enter_context(tc.tile_pool(name="atp", bufs=1))
        probsp = top.enter_context(tc.tile_pool(name="probsp", bufs=4))
        dsp = top.enter_context(tc.tile_pool(name="dsp", bufs=2))
        rbp = top.enter_context(tc.tile_pool(name="rbp", bufs=2))
        outp = top.enter_context(tc.tile_pool(name="outp", bufs=3))
        # PSUM: pps 2x1 + scp 2x2 + pvp 2x1 = 8 banks
        pps = top.enter_context(tc.tile_pool(name="pps", bufs=2, space="PSUM"))
        scp = top.enter_context(tc.tile_pool(name="scp", bufs=2, space="PSUM"))
        pvp = top.enter_context(tc.tile_pool(name="pvp", bufs=2, space="PSUM"))

        # ---- constants ----
        ebias = consts.tile([128, 2], F32, name="ebias")  # exp bias: [0]=0, [1]=ctx -2
        nc.vector.memset(ebias[:, 0:1], 0.0)
        nc.vector.memset(ebias[:, 1:2], -2.0)
        ones64 = consts.tile([128, 64], BF16, name="ones64")
        nc.vector.memset(ones64, 1.0)
        maskband = consts.tile([128, 256], BF16, name="maskband")
        bias_qk = consts.tile([128, 8], F32, name="bias_qk")
        bias_kc = consts.tile([128, 4], F32, name="bias_kc")
        bvb = consts.tile([128, 512], BF16, name="bvb")
        bvcb = consts.tile([128, 512], BF16, name="bvcb")

        # ---- SBUF activation/weight tiles ----
        ctx_sb = wts.tile([128, 8, Tc], BF16, name="ctx_sb")
        wkc_sb = wts.tile([128, 4, 8, 128], BF16, name="wkc_sb")
        wvc_sb = wts.tile([128, 8, 512], BF16, name="wvc_sb")
        wq_sb = wts.tile([128, 4, 8, 128], BF16, name="wq_sb")
        wk_sb = wts.tile([128, 4, 8, 128], BF16, name="wk_sb")
        wv_sb = wts.tile([128, 8, 512], BF16, name="wv_sb")
        wpj_sb = wts.tile([128, 4, 1024], BF16, name="wpj_sb")
        x0_sb = xp.tile([128, 8, 512], BF16, name="x0_sb")
        x1_sb = xp.tile([128, 8, 384], BF16, name="x1_sb")

        # ---- input loads ----
        # sync HWDGE queue: critical-path order
        nc.sync.dma_start(out=ctx_sb, in_=ctx_h[:, :, :])
        nc.sync.dma_start(out=wkc_sb[:, 0, :, :], in_=wkc_h[:, 0, :, :])
        nc.sync.dma_start(out=x0_sb[:, :, 0:256], in_=x0_h[:, :, 0:256])
        nc.sync.dma_start(out=wkc_sb[:, 1:4, :, :], in_=wkc_h[:, 1:4, :, :])
        nc.sync.dma_start(out=x0_sb[:, :, 256:512], in_=x0_h[:, :, 256:512])
        for f in range(4):
            nc.sync.dma_start(out=wq_sb[:, f, :, :], in_=wq_h[:, f, :, :])
            nc.sync.dma_start(out=wk_sb[:, f, :, :], in_=wk_h[:, f, :, :])
        nc.sync.dma_start(out=x1_sb, in_=x1_h[:, :, :])
        nc.sync.dma_start(out=wpj_sb, in_=wpj_h[:, :, :])
        # scalar HWDGE queue: consts first (unblock ctx-proj drains), then bulk v
        nc.scalar.dma_start(out=bias_kc, in_=bkc_h[:, :])
        nc.scalar.dma_start(out=bias_qk, in_=bqk_h[:, :])
        nc.scalar.dma_start(out=maskband, in_=mb_h[:, :])
        nc.scalar.dma_start(out=bvcb, in_=bvc_h[:, :])
        nc.scalar.dma_start(out=bvb, in_=bv_h[:, :])
        nc.scalar.dma_start(out=wv_sb[:, 0:4, :], in_=wv_h[:, 0:4, :])
        nc.scalar.dma_start(out=wv_sb[:, 4:8, :], in_=wv_h[:, 4:8, :])
        nc.scalar.dma_start(out=wvc_sb, in_=wvc_h[:, :, :])

        # ---- persistent activation tiles (token-chunk-split: clean deps) ----
        qT0 = [qkp.tile([128, 512], BF16, name=f"qT0_{p}") for p in range(NPAIR)]
        qT1 = [qkp.tile([128, 384], BF16, name=f"qT1_{p}") for p in range(NPAIR)]
        kTa = [qkp.tile([128, 640], BF16, name=f"kTa{p}") for p in range(NPAIR)]
        kTb = [qkp.tile([128, 384], BF16, name=f"kTb{p}") for p in range(NPAIR)]
        v_sb = [vtp.tile([128, 8, 64], BF16, name=f"v{c}") for c in range(8)]
        aT0 = [atp.tile([128, 512], BF16, name=f"aT0_{p}") for p in range(NPAIR)]
        aT1 = [atp.tile([128, 384], BF16, name=f"aT1_{p}") for p in range(NPAIR)]

        def kt_slice(p, c):
            """kT columns [128c, 128c+128) of pair p (ctx + k concatenated)."""
            if c <= 4:
                return kTa[p][:, 128 * c:128 * c + 128]
            return kTb[p][:, 128 * c - 640:128 * c - 512]

        def x_slice(kc, ts, te):
            if te <= 512:
                return x0_sb[:, kc, ts:te]
            return x1_sb[:, kc, ts - 512:te - 512]

        # ---- ctx projections: kcT -> kTa cols 0:128, vc -> v_sb[0] ----
        for f in range(4):
            pt = pps.tile([128, 512], F32, tag="pp", name=f"pkc{f}")
            for kc in range(8):
                nc.tensor.matmul(
                    pt[:, 0:Tc], wkc_sb[:, f, kc, :],
                    ctx_sb[:, kc, :], start=(kc == 0), stop=(kc == 7))
            nc.scalar.activation(
                out=kTa[f][:, 0:Tc], in_=pt[:, 0:Tc], func=AF.Identity,
                bias=bias_kc[:, f:f + 1], scale=1.0)
        # ---- v projection (natural layout) ----
        def v_proj(tt):
            pt = pps.tile([128, 512], F32, tag="pp", name=f"pv{tt}")
            for kc in range(8):
                nc.tensor.matmul(
                    pt[:, 0:512], x_slice(kc, 128 * tt, 128 * tt + 128),
                    wv_sb[:, kc, :], start=(kc == 0), stop=(kc == 7))
            nc.vector.tensor_add(
                out=v_sb[1 + tt][:, :, :],
                in0=pt[:, 0:512].rearrange("p (h d) -> p h d", h=8),
                in1=bvb.rearrange("p (h d) -> p h d", h=8))

        for tt in range(4):
            v_proj(tt)

        pt = pps.tile([128, 512], F32, tag="pp", name="pvc")
        for kc in range(8):
            nc.tensor.matmul(
                pt[:, 0:512], ctx_sb[:, kc, :], wvc_sb[:, kc, :],
                start=(kc == 0), stop=(kc == 7))
        nc.vector.tensor_add(
            out=v_sb[0][:, :, :],
            in0=pt[:, 0:512].rearrange("p (h d) -> p h d", h=8),
            in1=bvcb.rearrange("p (h d) -> p h d", h=8))

        # ---- qT / kT projections (transposed layout), per token chunk ----
        def qk_ftile(w_sb, f, dest, dcol, bias_col, ts, te, drain):
            pt = pps.tile([128, 512], F32, tag="pp", name=f"pqk{bias_col}{ts}")
            for kc in range(8):
                nc.tensor.matmul(
                    pt[:, 0:te - ts], w_sb[:, f, kc, :],
                    x_slice(kc, ts, te), start=(kc == 0), stop=(kc == 7))
            if drain == "act":
                nc.scalar.activation(
                    out=dest[:, dcol:dcol + te - ts], in_=pt[:, 0:te - ts],
                    func=AF.Identity, bias=bias_qk[:, bias_col:bias_col + 1],
                    scale=1.0)
            else:
                nc.vector.tensor_scalar_add(
                    out=dest[:, dcol:dcol + te - ts], in0=pt[:, 0:te - ts],
                    scalar1=bias_qk[:, bias_col:bias_col + 1])

        def attn(p, t_i):
            ts, te = TCH[t_i]
            n = te - ts
            last_c = 4 if t_i == 0 else 7
            qT = qT0[p] if t_i == 0 else qT1[p]
            aT = aT0[p] if t_i == 0 else aT1[p]
            pa = pvp.tile([128, 512], F32, tag="pa", name=f"pa{p}{t_i}")
            dsum = dsp.tile([128, 2, 512], BF16, tag="ds", name=f"ds{p}{t_i}")
            chunks = [c for c in range(8) if _rect(c, ts, te) is not None]

            def scores(c):
                cs, _ = _rect(c, ts, te)
                sc = scp.tile([128, 2, 512], F32, tag="sc", name=f"sc{p}{t_i}{c}")
                for hi in range(2):
                    nc.tensor.matmul(
                        sc[:, hi, cs - ts:n],
                        kt_slice(p, c)[64 * hi:64 * hi + 64, :],
                        qT[64 * hi:64 * hi + 64, cs - ts:n],
                        start=True, stop=True, tile_position=(64 * hi, 0))
                pb = probsp.tile([128, 2, 512], BF16, tag="pb", name=f"pb{p}{t_i}{c}")
                nc.scalar.activation(
                    out=pb[:, :, cs - ts:n], in_=sc[:, :, cs - ts:n],
                    func=AF.Exp,
                    bias=(ebias[:, 1:2] if c == 0 else ebias[:, 0:1]),
                    scale=0.125)
                for hi in range(2):
                    for s0, e0, mc in _band_pieces(c, ts, te):
                        nc.vector.tensor_mul(
                            out=pb[:, hi, s0 - ts:e0 - ts],
                            in0=pb[:, hi, s0 - ts:e0 - ts],
                            in1=maskband[:, mc:mc + (e0 - s0)])
                return pb

            def pv(c, pb):
                cs, _ = _rect(c, ts, te)
                for hi in range(2):
                    nc.tensor.matmul(
                        pa[64 * hi:64 * hi + 64, cs - ts:n],
                        v_sb[c][:, 2 * p + hi, :],
                        pb[:, hi, cs - ts:n],
                        start=(c == 0), stop=(c == last_c),
                        skip_group_check=True, tile_position=(0, 64 * hi))
                if c == 0:
                    nc.vector.tensor_copy(out=dsum[:, :, 0:n], in_=pb[:, :, 0:n])
                else:
                    nc.vector.tensor_add(
                        out=dsum[:, :, cs - ts:n], in0=dsum[:, :, cs - ts:n],
                        in1=pb[:, :, cs - ts:n])

            pending = None
            for c in chunks:
                pb = scores(c)
                if pending is not None:
                    pv(*pending)
                pending = (c, pb)
            pv(*pending)
            pd = scp.tile([128, 2, 512], F32, tag="sc", name=f"pd{p}{t_i}")
            for hi in range(2):
                nc.tensor.matmul(
                    pd[64 * hi:64 * hi + 64, 0, 0:n], ones64, dsum[:, hi, 0:n],
                    start=True, stop=True, tile_position=(0, 64 * hi),
                    skip_group_check=True)
            rb = rbp.tile([128, 512], F32, tag="rb", name=f"rb{p}{t_i}")
            nc.vector.reciprocal(out=rb[:, 0:n], in_=pd[:, 0, 0:n])
            nc.vector.tensor_mul(out=aT[:, 0:n], in0=pa[:, 0:n], in1=rb[:, 0:n])

        def outproj(t_i, ofs):
            ts, te = TCH[t_i]
            n = te - ts
            aT = aT0 if t_i == 0 else aT1
            for of in ofs:
                pt = pps.tile([128, 512], F32, tag="pp", name=f"po{of}{t_i}")
                for kc in range(4):
                    nc.tensor.matmul(
                        pt[:, 0:n], wpj_sb[:, kc, 128 * of:128 * of + 128],
                        aT[kc][:, 0:n], start=(kc == 0), stop=(kc == 3))
                # t0 drains on DVE (ACT busy with t1 exps); t1 drains on ACT
                # (free after the last exp, DVE busy with denominators).
                # out DMAs alternate between the two HWDGE queues.
                ob = outp.tile([128, 512], BF16, tag="ob", name=f"ob{of}{t_i}")
                if t_i == 0:
                    nc.vector.tensor_copy(out=ob[:, 0:n], in_=pt[:, 0:n])
                else:
                    nc.scalar.copy(out=ob[:, 0:n], in_=pt[:, 0:n])
                eng = nc.sync if of % 2 == 0 else nc.scalar
                eng.dma_start(out=out_h[128 * of:128 * of + 128, ts:te], in_=ob[:, 0:n])

        def outproj_final(t_i):
            # of-pairs on scp-pool tiles (scores are done; 4 of-tiles in flight)
            ts, te = TCH[t_i]
            n = te - ts
            aT = aT0 if t_i == 0 else aT1
            for og in range(4):
                pt = scp.tile([128, 2, 512], F32, tag="sc", name=f"pg{og}{t_i}")
                for i in range(2):
                    of = 2 * og + i
                    for kc in range(4):
                        nc.tensor.matmul(
                            pt[:, i, 0:n], wpj_sb[:, kc, 128 * of:128 * of + 128],
                            aT[kc][:, 0:n], start=(kc == 0), stop=(kc == 3))
                ob = outp.tile([128, 2, 512], BF16, tag="obg", name=f"obg{og}{t_i}")
                nc.vector.tensor_copy(out=ob[0:64, :, 0:n], in_=pt[0:64, :, 0:n])
                nc.scalar.copy(out=ob[64:128, :, 0:n], in_=pt[64:128, :, 0:n])
                for i in range(2):
                    of = 2 * og + i
                    nc.sync.dma_start(
                        out=out_h[128 * of:128 * of + 64, ts:te], in_=ob[0:64, i, 0:n])
                    nc.scalar.dma_start(
                        out=out_h[128 * of + 64:128 * of + 128, ts:te],
                        in_=ob[64:128, i, 0:n])

        # ---- schedule: pair-major pipeline; outproj fills trailing stalls ----
        for p in range(NPAIR):
            qk_ftile(wq_sb, p, qT0[p], 0, p, 0, 512, "act")
            qk_ftile(wk_sb, p, kTa[p], Tc, 4 + p, 0, 512, "dve")
            attn(p, 0)
        for tt in range(4, 7):
            v_proj(tt)
        for p in range(NPAIR):
            qk_ftile(wq_sb, p, qT1[p], 0, p, 512, 896, "act")
            qk_ftile(wk_sb, p, kTb[p], 0, 4 + p, 512, 896, "dve")
            attn(p, 1)
            outproj(0, range(2 * p, 2 * p + 2))
        outproj(1, range(8))

    if not nc.is_finalized():
        nc.finalize()
    return nc


_NC_CACHE = {}


def _get_nc():
    if "nc" not in _NC_CACHE:
        _NC_CACHE["nc"] = build_nc()
    return _NC_CACHE["nc"]


def _pack128(v):
    """[128*n] -> [128, n] with [p, f] = v[128*f + p]."""
    n = v.shape[0] // 128
    return np.ascontiguousarray(v.reshape(n, 128).T)


def _kc_major(w):
    """[1024, F] -> [128, 8, F]: partition-major with kc chunks."""
    F = w.shape[1]
    return np.ascontiguousarray(w.reshape(8, 128, F).transpose(1, 0, 2))


def _f_major(w):
    """[1024, 512] -> [128, 4 fblk, 8 kc, 128]."""
    r = w.reshape(8, 128, 4, 128)  # kc, p, f, ff
    return np.ascontiguousarray(r.transpose(1, 2, 0, 3))


def make_in_maps(inputs):
    bf16 = ml_dtypes.bfloat16
    x = np.asarray(inputs["x"], np.float32)
    ctx_seq = np.asarray(inputs["context_seq"], np.float32)
    w_ref = np.asarray(inputs["w_ref"], np.float32)
    b_ref = np.asarray(inputs["b_ref"], np.float32)
    w_attn = np.asarray(inputs["w_attn"], np.float32)
    b_attn = np.asarray(inputs["b_attn"], np.float32)
    w_proj = np.asarray(inputs["w_proj"], np.float32)

    # mask band constant: cols 0-127 causal (1 where q>=p), cols 128-255
    # anti-diagonal (0 where q==p else 1)
    qq = np.arange(128)[None, :]
    pp = np.arange(128)[:, None]
    mband = np.ascontiguousarray(
        np.concatenate([(qq >= pp), (qq != pp)], axis=1).astype(bf16))

    in_maps = []
    for b in range(4):
        xT = x[b].T.astype(bf16)  # [1024, 896]
        x0r = _kc_major(xT[:, 0:512])
        x1r = _kc_major(xT[:, 512:896])
        ctxr = _kc_major(ctx_seq[b].T.astype(bf16))
        for g in range(2):
            sl = slice(512 * g, 512 * g + 512)
            in_maps.append(dict(
                x0r=x0r,
                x1r=x1r,
                ctxr=ctxr,
                w_q=_f_major(w_attn[:, 0 * NX:1 * NX][:, sl].astype(bf16)),
                w_k=_f_major(w_attn[:, 1 * NX:2 * NX][:, sl].astype(bf16)),
                w_v=_kc_major(w_attn[:, 2 * NX:3 * NX][:, sl].astype(bf16)),
                w_kc=_f_major(w_ref[:, 0 * NX:1 * NX][:, sl].astype(bf16)),
                w_vc=_kc_major(w_ref[:, 1 * NX:2 * NX][:, sl].astype(bf16)),
                w_pj=np.ascontiguousarray(
                    w_proj[sl, :].astype(bf16).reshape(4, 128, NX).transpose(1, 0, 2)),
                b_qk=_pack128(np.concatenate([b_attn[0 * NX:1 * NX][sl],
                                              b_attn[1 * NX:2 * NX][sl]])),
                b_kc=_pack128(b_ref[0 * NX:1 * NX][sl]),
                b_v=np.ascontiguousarray(np.broadcast_to(
                    b_attn[2 * NX:3 * NX][sl].astype(bf16), (128, 512))),
                b_vc=np.ascontiguousarray(np.broadcast_to(
                    b_ref[1 * NX:2 * NX][sl].astype(bf16), (128, 512))),
                mband=mband,
            ))
    return in_maps


def kernel(**inputs):
    b_proj = np.asarray(inputs["b_proj"], np.float32)
    in_maps = make_in_maps(inputs)
    nc = _get_nc()
    res = run_bass_kernel_spmd(nc, in_maps, core_ids=list(range(8)),
                               trace=os.environ.get("COCON_TRACE", "") == "1")
    outs = res.results
    out = np.empty((4, T, NX), np.float32)
    for b in range(4):
        acc = (outs[2 * b]["outT"].astype(np.float32)
               + outs[2 * b + 1]["outT"].astype(np.float32))  # [1024, 896]
        out[b] = acc.T + b_proj[None, :]
    if res.exec_time_ns is not None:
        kernel.last_exec_time_ns = res.exec_time_ns
    return out


kernel.last_exec_time_ns = None


# revision 5
# speedup vs baseline: 1.1000x; 1.0057x over previous
"""Trainium2 Bass kernel for nn_CoconAttention (dense transformer attention block).

Sharding: 8 cores = 4 batches x 2 head-groups (8 heads each). Each core gets
pre-transposed/sliced bf16 inputs (host pre-arranges every tensor into its
exact on-chip layout so all DMAs are contiguous), computes its partial output
outT [1024, 896] (bf16, transposed, pre-b_proj), and the host sums head-group
pairs + transposes.

Per core (H=8 heads, Dh=64, T=896, Tc=128, S=1024), bf16 compute / fp32 PSUM:
  qT/kT      : feature-major head-pair tiles (2 heads x 64 rows), split per
               token chunk (qT0/qT1) and ctx|t0 / t1 (kTa/kTb)
  scores^T   : [128 keys, 2 heads, tok] psum; exp on ACT -> bf16 probs
  probs^T    : masked via precomputed band masks (DVE mult), summed into dsum
  PV         : col-tiled matmuls, head hi -> psum partitions 64*hi..64*hi+64
  denom      : dsum (DVE bf16 accum over chunks) then ones[128,64]-stationary
               matmul -> denominator replicated across 64 partitions per head
  aT         : normalized via DVE reciprocal+mult, bf16
  out-proj   : per token-chunk, interleaved with the other chunk's attention
"""
import os
import sys

import numpy as np
import ml_dtypes

try:
    import concourse.bass as bass
except ImportError:  # fresh grading dir: fall back to the repo location
    sys.path.insert(0, "/opt/trn_rl_repo")
    import concourse.bass as bass
import concourse.bacc as bacc

import concourse.tile as tile
from concourse import mybir
from concourse.bass_utils import run_bass_kernel_spmd
from contextlib import ExitStack

F32 = mybir.dt.float32
BF16 = mybir.dt.bfloat16
AF = mybir.ActivationFunctionType

T, Tc, NX = 896, 128, 1024
TCH = ((0, 512), (512, 896))  # tok chunks
NPAIR = 4  # head pairs per core


def _rect(c, ts, te):
    """Live (unmasked) column range of scores chunk c within tok range [ts,te)."""
    cs = max(max(0, 128 * (c - 1)), ts)
    return None if cs >= te else (cs, te)


def _band_pieces(c, ts, te):
    """Mask applications for chunk c in [ts,te): (s0, e0, mask_col_offset)."""
    if c == 0:
        bs, be, moff, borig = 0, 128, 128, 0  # diag half only
    elif c <= 6:
        bs = 128 * (c - 1)
        be, moff, borig = bs + 256, 0, bs  # causal(128) + diag(128)
    else:
        bs, be, moff, borig = 768, 896, 0, 768  # causal half only
    s0, e0 = max(bs, ts), min(be, te)
    if s0 >= e0:
        return []
    return [(s0, e0, moff + (s0 - borig))]


def build_nc():
    nc = bacc.Bacc("TRN2", target_bir_lowering=False)

    # host pre-arranged layouts (partition-major, fully contiguous loads)
    x0_h = nc.dram_tensor("x0r", [128, 8, 512], BF16, kind="ExternalInput")
    x1_h = nc.dram_tensor("x1r", [128, 8, 384], BF16, kind="ExternalInput")
    ctx_h = nc.dram_tensor("ctxr", [128, 8, Tc], BF16, kind="ExternalInput")
    wq_h = nc.dram_tensor("w_q", [128, 4, 8, 128], BF16, kind="ExternalInput")
    wk_h = nc.dram_tensor("w_k", [128, 4, 8, 128], BF16, kind="ExternalInput")
    wv_h = nc.dram_tensor("w_v", [128, 8, 512], BF16, kind="ExternalInput")
    wkc_h = nc.dram_tensor("w_kc", [128, 4, 8, 128], BF16, kind="ExternalInput")
    wvc_h = nc.dram_tensor("w_vc", [128, 8, 512], BF16, kind="ExternalInput")
    wpj_h = nc.dram_tensor("w_pj", [128, 4, 1024], BF16, kind="ExternalInput")
    bqk_h = nc.dram_tensor("b_qk", [128, 8], F32, kind="ExternalInput")
    bkc_h = nc.dram_tensor("b_kc", [128, 4], F32, kind="ExternalInput")
    bv_h = nc.dram_tensor("b_v", [128, 512], BF16, kind="ExternalInput")
    bvc_h = nc.dram_tensor("b_vc", [128, 512], BF16, kind="ExternalInput")
    mb_h = nc.dram_tensor("mband", [128, 256], BF16, kind="ExternalInput")
    out_h = nc.dram_tensor("outT", [NX, T], BF16, kind="ExternalOutput")

    with tile.TileContext(nc) as tc, ExitStack() as top:
        consts = top.enter_context(tc.tile_pool(name="consts", bufs=1))
        wts = top.enter_context(tc.tile_pool(name="wts", bufs=1))
        xp = top.enter_context(tc.tile_pool(name="xp", bufs=1))
        qkp = top.enter_context(tc.tile_pool(name="qkp", bufs=1))
        vtp = top.enter_context(tc.tile_pool(name="vtp", bufs=1))
        atp = top.enter_context(tc.tile_pool(name="atp", bufs=1))
        probsp = top.enter_context(tc.tile_pool(name="probsp", bufs=4))
        dsp = top.enter_context(tc.tile_pool(name="dsp", bufs=3))
        rbp = top.enter_context(tc.tile_pool(name="rbp", bufs=3))
        outp = top.enter_context(tc.tile_pool(name="outp", bufs=4))
        # PSUM: pps 2x1 + scp 2x2 + pvp 2x1 = 8 banks
        pps = top.enter_context(tc.tile_pool(name="pps", bufs=2, space="PSUM"))
        scp = top.enter_context(tc.tile_pool(name="scp", bufs=2, space="PSUM"))
        pvp = top.enter_context(tc.tile_pool(name="pvp", bufs=2, space="PSUM"))

        # ---- constants ----
        ebias = consts.tile([128, 2], F32, name="ebias")  # exp bias: [0]=0, [1]=ctx -2
        nc.vector.memset(ebias[:, 0:1], 0.0)
        nc.vector.memset(ebias[:, 1:2], -2.0)
        ones64 = consts.tile([128, 64], BF16, name="ones64")
        nc.vector.memset(ones64, 1.0)
        maskband = consts.tile([128, 256], BF16, name="maskband")
        bias_qk = consts.tile([128, 8], F32, name="bias_qk")
        bias_kc = consts.tile([128, 4], F32, name="bias_kc")
        bvb = consts.tile([128, 512], BF16, name="bvb")
        bvcb = consts.tile([128, 512], BF16, name="bvcb")

        # ---- SBUF activation/weight tiles ----
        ctx_sb = wts.tile([128, 8, Tc], BF16, name="ctx_sb")
        wkc_sb = wts.tile([128, 4, 8, 128], BF16, name="wkc_sb")
        wvc_sb = wts.tile([128, 8, 512], BF16, name="wvc_sb")
        wq_sb = wts.tile([128, 4, 8, 128], BF16, name="wq_sb")
        wk_sb = wts.tile([128, 4, 8, 128], BF16, name="wk_sb")
        wv_sb = wts.tile([128, 8, 512], BF16, name="wv_sb")
        wpj_sb = wts.tile([128, 4, 1024], BF16, name="wpj_sb")
        x0_sb = xp.tile([128, 8, 512], BF16, name="x0_sb")
        x1_sb = xp.tile([128, 8, 384], BF16, name="x1_sb")

        # ---- input loads ----
        # sync HWDGE queue: critical-path order
        nc.sync.dma_start(out=ctx_sb, in_=ctx_h[:, :, :])
        nc.sync.dma_start(out=wkc_sb[:, 0, :, :], in_=wkc_h[:, 0, :, :])
        nc.sync.dma_start(out=x0_sb[:, :, 0:256], in_=x0_h[:, :, 0:256])
        nc.sync.dma_start(out=wkc_sb[:, 1:4, :, :], in_=wkc_h[:, 1:4, :, :])
        nc.sync.dma_start(out=x0_sb[:, :, 256:512], in_=x0_h[:, :, 256:512])
        for f in range(4):
            nc.sync.dma_start(out=wq_sb[:, f, :, :], in_=wq_h[:, f, :, :])
            nc.sync.dma_start(out=wk_sb[:, f, :, :], in_=wk_h[:, f, :, :])
        nc.sync.dma_start(out=x1_sb, in_=x1_h[:, :, :])
        nc.sync.dma_start(out=wpj_sb, in_=wpj_h[:, :, :])
        # scalar HWDGE queue: consts first (unblock ctx-proj drains), then bulk v
        nc.scalar.dma_start(out=bias_kc, in_=bkc_h[:, :])
        nc.scalar.dma_start(out=bias_qk, in_=bqk_h[:, :])
        nc.scalar.dma_start(out=maskband, in_=mb_h[:, :])
        nc.scalar.dma_start(out=bvcb, in_=bvc_h[:, :])
        nc.scalar.dma_start(out=bvb, in_=bv_h[:, :])
        nc.scalar.dma_start(out=wv_sb[:, 0:4, :], in_=wv_h[:, 0:4, :])
        nc.scalar.dma_start(out=wv_sb[:, 4:8, :], in_=wv_h[:, 4:8, :])
        nc.scalar.dma_start(out=wvc_sb, in_=wvc_h[:, :, :])

        # ---- persistent activation tiles (token-chunk-split: clean deps) ----
        qT0 = [qkp.tile([128, 512], BF16, name=f"qT0_{p}") for p in range(NPAIR)]
        qT1 = [qkp.tile([128, 384], BF16, name=f"qT1_{p}") for p in range(NPAIR)]
        kTa = [qkp.tile([128, 640], BF16, name=f"kTa{p}") for p in range(NPAIR)]
        kTb = [qkp.tile([128, 384], BF16, name=f"kTb{p}") for p in range(NPAIR)]
        v_sb = [vtp.tile([128, 8, 64], BF16, name=f"v{c}") for c in range(8)]
        aT0 = [atp.tile([128, 512], BF16, name=f"aT0_{p}") for p in range(NPAIR)]
        aT1 = [atp.tile([128, 384], BF16, name=f"aT1_{p}") for p in range(NPAIR)]

        def kt_slice(p, c):
            """kT columns [128c, 128c+128) of pair p (ctx + k concatenated)."""
            if c <= 4:
                return kTa[p][:, 128 * c:128 * c + 128]
            return kTb[p][:, 128 * c - 640:128 * c - 512]

        def x_slice(kc, ts, te):
            if te <= 512:
                return x0_sb[:, kc, ts:te]
            return x1_sb[:, kc, ts - 512:te - 512]

        # ---- ctx projections: kcT -> kTa cols 0:128, vc -> v_sb[0] ----
        for f in range(4):
            pt = pps.tile([128, 512], F32, tag="pp", name=f"pkc{f}")
            for kc in range(8):
                nc.tensor.matmul(
                    pt[:, 0:Tc], wkc_sb[:, f, kc, :],
                    ctx_sb[:, kc, :], start=(kc == 0), stop=(kc == 7))
            nc.scalar.activation(
                out=kTa[f][:, 0:Tc], in_=pt[:, 0:Tc], func=AF.Identity,
                bias=bias_kc[:, f:f + 1], scale=1.0)
        # ---- v projection (natural layout) ----
        def v_proj(tt):
            pt = pps.tile([128, 512], F32, tag="pp", name=f"pv{tt}")
            for kc in range(8):
                nc.tensor.matmul(
                    pt[:, 0:512], x_slice(kc, 128 * tt, 128 * tt + 128),
                    wv_sb[:, kc, :], start=(kc == 0), stop=(kc == 7))
            nc.vector.tensor_add(
                out=v_sb[1 + tt][:, :, :],
                in0=pt[:, 0:512].rearrange("p (h d) -> p h d", h=8),
                in1=bvb.rearrange("p (h d) -> p h d", h=8))

        for tt in range(4):
            v_proj(tt)

        pt = pps.tile([128, 512], F32, tag="pp", name="pvc")
        for kc in range(8):
            nc.tensor.matmul(
                pt[:, 0:512], ctx_sb[:, kc, :], wvc_sb[:, kc, :],
                start=(kc == 0), stop=(kc == 7))
        nc.vector.tensor_add(
            out=v_sb[0][:, :, :],
            in0=pt[:, 0:512].rearrange("p (h d) -> p h d", h=8),
            in1=bvcb.rearrange("p (h d) -> p h d", h=8))

        # ---- qT / kT projections (transposed layout), per token chunk ----
        def qk_ftile(w_sb, f, dest, dcol, bias_col, ts, te, drain):
            pt = pps.tile([128, 512], F32, tag="pp", name=f"pqk{bias_col}{ts}")
            for kc in range(8):
                nc.tensor.matmul(
                    pt[:, 0:te - ts], w_sb[:, f, kc, :],
                    x_slice(kc, ts, te), start=(kc == 0), stop=(kc == 7))
            if drain == "act":
                nc.scalar.activation(
                    out=dest[:, dcol:dcol + te - ts], in_=pt[:, 0:te - ts],
                    func=AF.Identity, bias=bias_qk[:, bias_col:bias_col + 1],
                    scale=1.0)
            else:
                nc.vector.tensor_scalar_add(
                    out=dest[:, dcol:dcol + te - ts], in0=pt[:, 0:te - ts],
                    scalar1=bias_qk[:, bias_col:bias_col + 1])

        def attn(p, t_i):
            ts, te = TCH[t_i]
            n = te - ts
            last_c = 4 if t_i == 0 else 7
            qT = qT0[p] if t_i == 0 else qT1[p]
            aT = aT0[p] if t_i == 0 else aT1[p]
            pa = pvp.tile([128, 512], F32, tag="pa", name=f"pa{p}{t_i}")
            dsum = dsp.tile([128, 2, 512], BF16, tag="ds", name=f"ds{p}{t_i}")
            chunks = [c for c in range(8) if _rect(c, ts, te) is not None]

            def scores(c):
                cs, _ = _rect(c, ts, te)
                sc = scp.tile([128, 2, 512], F32, tag="sc", name=f"sc{p}{t_i}{c}")
                for hi in range(2):
                    nc.tensor.matmul(
                        sc[:, hi, cs - ts:n],
                        kt_slice(p, c)[64 * hi:64 * hi + 64, :],
                        qT[64 * hi:64 * hi + 64, cs - ts:n],
                        start=True, stop=True, tile_position=(64 * hi, 0))
                pb = probsp.tile([128, 2, 512], BF16, tag="pb", name=f"pb{p}{t_i}{c}")
                nc.scalar.activation(
                    out=pb[:, :, cs - ts:n], in_=sc[:, :, cs - ts:n],
                    func=AF.Exp,
                    bias=(ebias[:, 1:2] if c == 0 else ebias[:, 0:1]),
                    scale=0.125)
                for hi in range(2):
                    for s0, e0, mc in _band_pieces(c, ts, te):
                        nc.vector.tensor_mul(
                            out=pb[:, hi, s0 - ts:e0 - ts],
                            in0=pb[:, hi, s0 - ts:e0 - ts],
                            in1=maskband[:, mc:mc + (e0 - s0)])
                return pb

            def pv(c, pb):
                cs, _ = _rect(c, ts, te)
                for hi in range(2):
                    nc.tensor.matmul(
                        pa[64 * hi:64 * hi + 64, cs - ts:n],
                        v_sb[c][:, 2 * p + hi, :],
                        pb[:, hi, cs - ts:n],
                        start=(c == 0), stop=(c == last_c),
                        skip_group_check=True, tile_position=(0, 64 * hi))
                if c == 0:
                    nc.vector.tensor_copy(out=dsum[:, :, 0:n], in_=pb[:, :, 0:n])
                else:
                    nc.vector.tensor_add(
                        out=dsum[:, :, cs - ts:n], in0=dsum[:, :, cs - ts:n],
                        in1=pb[:, :, cs - ts:n])

            pending = None
            for c in chunks:
                pb = scores(c)
                if pending is not None:
                    pv(*pending)
                pending = (c, pb)
            pv(*pending)
            pd = scp.tile([128, 2, 512], F32, tag="sc", name=f"pd{p}{t_i}")
            for hi in range(2):
                nc.tensor.matmul(
                    pd[64 * hi:64 * hi + 64, 0, 0:n], ones64, dsum[:, hi, 0:n],
                    start=True, stop=True, tile_position=(0, 64 * hi),
                    skip_group_check=True)
            rb = rbp.tile([128, 512], F32, tag="rb", name=f"rb{p}{t_i}")
            nc.vector.reciprocal(out=rb[:, 0:n], in_=pd[:, 0, 0:n])
            nc.vector.tensor_mul(out=aT[:, 0:n], in0=pa[:, 0:n], in1=rb[:, 0:n])

        def outproj(t_i, ofs):
            ts, te = TCH[t_i]
            n = te - ts
            aT = aT0 if t_i == 0 else aT1
            for of in ofs:
                pt = pps.tile([128, 512], F32, tag="pp", name=f"po{of}{t_i}")
                for kc in range(4):
                    nc.tensor.matmul(
                        pt[:, 0:n], wpj_sb[:, kc, 128 * of:128 * of + 128],
                        aT[kc][:, 0:n], start=(kc == 0), stop=(kc == 3))
                # t0 drains on DVE (ACT busy with t1 exps); t1 drains on ACT
                # (free after the last exp, DVE busy with denominators).
                # out DMAs alternate between the two HWDGE queues.
                ob = outp.tile([128, 512], BF16, tag="ob", name=f"ob{of}{t_i}")
                if t_i == 0 or of == 7:
                    nc.vector.tensor_copy(out=ob[:, 0:n], in_=pt[:, 0:n])
                else:
                    nc.scalar.copy(out=ob[:, 0:n], in_=pt[:, 0:n])
                eng = nc.sync if of % 2 == 0 else nc.scalar
                if t_i == 1 and of == 7:
                    eng = nc.sync
                eng.dma_start(out=out_h[128 * of:128 * of + 128, ts:te], in_=ob[:, 0:n])

        def outproj_final(t_i):
            # of-pairs on scp-pool tiles (scores are done; 4 of-tiles in flight)
            ts, te = TCH[t_i]
            n = te - ts
            aT = aT0 if t_i == 0 else aT1
            for og in range(4):
                pt = scp.tile([128, 2, 512], F32, tag="sc", name=f"pg{og}{t_i}")
                for i in range(2):
                    of = 2 * og + i
                    for kc in range(4):
                        nc.tensor.matmul(
                            pt[:, i, 0:n], wpj_sb[:, kc, 128 * of:128 * of + 128],
                            aT[kc][:, 0:n], start=(kc == 0), stop=(kc == 3))
                ob = outp.tile([128, 2, 512], BF16, tag="obg", name=f"obg{og}{t_i}")
                nc.vector.tensor_copy(out=ob[0:64, :, 0:n], in_=pt[0:64, :, 0:n])
                nc.scalar.copy(out=ob[64:128, :, 0:n], in_=pt[64:128, :, 0:n])
                for i in range(2):
                    of = 2 * og + i
                    nc.sync.dma_start(
                        out=out_h[128 * of:128 * of + 64, ts:te], in_=ob[0:64, i, 0:n])
                    nc.scalar.dma_start(
                        out=out_h[128 * of + 64:128 * of + 128, ts:te],
                        in_=ob[64:128, i, 0:n])

        # ---- schedule: pair-major pipeline; outproj fills trailing stalls ----
        for p in range(NPAIR):
            qk_ftile(wq_sb, p, qT0[p], 0, p, 0, 512, "act")
            qk_ftile(wk_sb, p, kTa[p], Tc, 4 + p, 0, 512, "dve")
            attn(p, 0)
        for tt in range(4, 7):
            v_proj(tt)
        for p in range(NPAIR):
            qk_ftile(wq_sb, p, qT1[p], 0, p, 512, 896, "act")
            qk_ftile(wk_sb, p, kTb[p], 0, 4 + p, 512, 896, "dve")
            attn(p, 1)
            outproj(0, range(2 * p, 2 * p + 2))
        outproj(1, range(8))

    if not nc.is_finalized():
        nc.finalize()
    return nc


_NC_CACHE = {}


def _get_nc():
    if "nc" not in _NC_CACHE:
        _NC_CACHE["nc"] = build_nc()
    return _NC_CACHE["nc"]


def _pack128(v):
    """[128*n] -> [128, n] with [p, f] = v[128*f + p]."""
    n = v.shape[0] // 128
    return np.ascontiguousarray(v.reshape(n, 128).T)


def _kc_major(w):
    """[1024, F] -> [128, 8, F]: partition-major with kc chunks."""
    F = w.shape[1]
    return np.ascontiguousarray(w.reshape(8, 128, F).transpose(1, 0, 2))


def _f_major(w):
    """[1024, 512] -> [128, 4 fblk, 8 kc, 128]."""
    r = w.reshape(8, 128, 4, 128)  # kc, p, f, ff
    return np.ascontiguousarray(r.transpose(1, 2, 0, 3))


def make_in_maps(inputs):
    bf16 = ml_dtypes.bfloat16
    x = np.asarray(inputs["x"], np.float32)
    ctx_seq = np.asarray(inputs["context_seq"], np.float32)
    w_ref = np.asarray(inputs["w_ref"], np.float32)
    b_ref = np.asarray(inputs["b_ref"], np.float32)
    w_attn = np.asarray(inputs["w_attn"], np.float32)
    b_attn = np.asarray(inputs["b_attn"], np.float32)
    w_proj = np.asarray(inputs["w_proj"], np.float32)

    # mask band constant: cols 0-127 causal (1 where q>=p), cols 128-255
    # anti-diagonal (0 where q==p else 1)
    qq = np.arange(128)[None, :]
    pp = np.arange(128)[:, None]
    mband = np.ascontiguousarray(
        np.concatenate([(qq >= pp), (qq != pp)], axis=1).astype(bf16))

    in_maps = []
    for b in range(4):
        xT = x[b].T.astype(bf16)  # [1024, 896]
        x0r = _kc_major(xT[:, 0:512])
        x1r = _kc_major(xT[:, 512:896])
        ctxr = _kc_major(ctx_seq[b].T.astype(bf16))
        for g in range(2):
            sl = slice(512 * g, 512 * g + 512)
            in_maps.append(dict(
                x0r=x0r,
                x1r=x1r,
                ctxr=ctxr,
                w_q=_f_major(w_attn[:, 0 * NX:1 * NX][:, sl].astype(bf16)),
                w_k=_f_major(w_attn[:, 1 * NX:2 * NX][:, sl].astype(bf16)),
                w_v=_kc_major(w_attn[:, 2 * NX:3 * NX][:, sl].astype(bf16)),
                w_kc=_f_major(w_ref[:, 0 * NX:1 * NX][:, sl].astype(bf16)),
                w_vc=_kc_major(w_ref[:, 1 * NX:2 * NX][:, sl].astype(bf16)),
                w_pj=np.ascontiguousarray(
                    w_proj[sl, :].astype(bf16).reshape(4, 128, NX).transpose(1, 0, 2)),
                b_qk=_pack128(np.concatenate([b_attn[0 * NX:1 * NX][sl],
                                              b_attn[1 * NX:2 * NX][sl]])),
                b_kc=_pack128(b_ref[0 * NX:1 * NX][sl]),
                b_v=np.ascontiguousarray(np.broadcast_to(
                    b_attn[2 * NX:3 * NX][sl].astype(bf16), (128, 512))),
                b_vc=np.ascontiguousarray(np.broadcast_to(
                    b_ref[1 * NX:2 * NX][sl].astype(bf16), (128, 512))),
                mband=mband,
            ))
    return in_maps


def kernel(**inputs):
    b_proj = np.asarray(inputs["b_proj"], np.float32)
    in_maps = make_in_maps(inputs)
    nc = _get_nc()
    res = run_bass_kernel_spmd(nc, in_maps, core_ids=list(range(8)),
                               trace=os.environ.get("COCON_TRACE", "") == "1")
    outs = res.results
    out = np.empty((4, T, NX), np.float32)
    for b in range(4):
        acc = (outs[2 * b]["outT"].astype(np.float32)
               + outs[2 * b + 1]["outT"].astype(np.float32))  # [1024, 896]
        out[b] = acc.T + b_proj[None, :]
    if res.exec_time_ns is not None:
        kernel.last_exec_time_ns = res.exec_time_ns
    return out


kernel.last_exec_time_ns = None


# revision 6
# speedup vs baseline: 1.1075x; 1.0068x over previous
"""Trainium2 Bass kernel for nn_CoconAttention (dense transformer attention block).

Sharding: 8 cores = 4 batches x 2 head-groups (8 heads each). Each core gets
pre-transposed/sliced bf16 inputs (host pre-arranges every tensor into its
exact on-chip layout so all DMAs are contiguous), computes its partial output
outT [1024, 896] (bf16, transposed, pre-b_proj), and the host sums head-group
pairs + transposes.

Per core (H=8 heads, Dh=64, T=896, Tc=128, S=1024), bf16 compute / fp32 PSUM:
  qT/kT      : feature-major head-pair tiles (2 heads x 64 rows), split per
               token chunk (qT0/qT1) and ctx|t0 / t1 (kTa/kTb)
  scores^T   : [128 keys, 2 heads, tok] psum; exp on ACT -> bf16 probs
  probs^T    : masked via precomputed band masks (DVE mult), summed into dsum
  PV         : col-tiled matmuls, head hi -> psum partitions 64*hi..64*hi+64
  denom      : dsum (DVE bf16 accum over chunks) then ones[128,64]-stationary
               matmul -> denominator replicated across 64 partitions per head
  aT         : normalized via DVE reciprocal+mult, bf16
  out-proj   : per token-chunk, interleaved with the other chunk's attention
"""
import os
import sys

import numpy as np
import ml_dtypes

try:
    import concourse.bass as bass
except ImportError:  # fresh grading dir: fall back to the repo location
    sys.path.insert(0, "/opt/trn_rl_repo")
    import concourse.bass as bass
import concourse.bacc as bacc

import concourse.tile as tile
from concourse import mybir
from concourse.bass_utils import run_bass_kernel_spmd
from contextlib import ExitStack

F32 = mybir.dt.float32
BF16 = mybir.dt.bfloat16
AF = mybir.ActivationFunctionType

T, Tc, NX = 896, 128, 1024
TCH = ((0, 512), (512, 896))  # tok chunks
NPAIR = 4  # head pairs per core


def _rect(c, ts, te):
    """Live (unmasked) column range of scores chunk c within tok range [ts,te)."""
    cs = max(max(0, 128 * (c - 1)), ts)
    return None if cs >= te else (cs, te)


def _band_pieces(c, ts, te):
    """Mask applications for chunk c in [ts,te): (s0, e0, mask_col_offset)."""
    if c == 0:
        bs, be, moff, borig = 0, 128, 128, 0  # diag half only
    elif c <= 6:
        bs = 128 * (c - 1)
        be, moff, borig = bs + 256, 0, bs  # causal(128) + diag(128)
    else:
        bs, be, moff, borig = 768, 896, 0, 768  # causal half only
    s0, e0 = max(bs, ts), min(be, te)
    if s0 >= e0:
        return []
    return [(s0, e0, moff + (s0 - borig))]


def build_nc():
    nc = bacc.Bacc("TRN2", target_bir_lowering=False)

    # host pre-arranged layouts (partition-major, fully contiguous loads)
    x0_h = nc.dram_tensor("x0r", [128, 8, 512], BF16, kind="ExternalInput")
    x1_h = nc.dram_tensor("x1r", [128, 8, 384], BF16, kind="ExternalInput")
    ctx_h = nc.dram_tensor("ctxr", [128, 8, Tc], BF16, kind="ExternalInput")
    wq_h = nc.dram_tensor("w_q", [128, 4, 8, 128], BF16, kind="ExternalInput")
    wk_h = nc.dram_tensor("w_k", [128, 4, 8, 128], BF16, kind="ExternalInput")
    wv_h = nc.dram_tensor("w_v", [128, 8, 512], BF16, kind="ExternalInput")
    wkc_h = nc.dram_tensor("w_kc", [128, 4, 8, 128], BF16, kind="ExternalInput")
    wvc_h = nc.dram_tensor("w_vc", [128, 8, 512], BF16, kind="ExternalInput")
    wpj_h = nc.dram_tensor("w_pj", [128, 4, 1024], BF16, kind="ExternalInput")
    bqk_h = nc.dram_tensor("b_qk", [128, 8], F32, kind="ExternalInput")
    bkc_h = nc.dram_tensor("b_kc", [128, 4], F32, kind="ExternalInput")
    bv_h = nc.dram_tensor("b_v", [128, 512], BF16, kind="ExternalInput")
    bvc_h = nc.dram_tensor("b_vc", [128, 512], BF16, kind="ExternalInput")
    mb_h = nc.dram_tensor("mband", [128, 256], BF16, kind="ExternalInput")
    out_h = nc.dram_tensor("outT", [NX, T], BF16, kind="ExternalOutput")

    with tile.TileContext(nc) as tc, ExitStack() as top:
        consts = top.enter_context(tc.tile_pool(name="consts", bufs=1))
        wts = top.enter_context(tc.tile_pool(name="wts", bufs=1))
        xp = top.enter_context(tc.tile_pool(name="xp", bufs=1))
        qkp = top.enter_context(tc.tile_pool(name="qkp", bufs=1))
        vtp = top.enter_context(tc.tile_pool(name="vtp", bufs=1))
        atp = top.enter_context(tc.tile_pool(name="atp", bufs=1))
        probsp = top.enter_context(tc.tile_pool(name="probsp", bufs=4))
        dsp = top.enter_context(tc.tile_pool(name="dsp", bufs=3))
        rbp = top.enter_context(tc.tile_pool(name="rbp", bufs=3))
        outp = top.enter_context(tc.tile_pool(name="outp", bufs=4))
        # PSUM: pps 2x1 + scp 2x2 + pvp 2x1 = 8 banks
        pps = top.enter_context(tc.tile_pool(name="pps", bufs=2, space="PSUM"))
        scp = top.enter_context(tc.tile_pool(name="scp", bufs=2, space="PSUM"))
        pvp = top.enter_context(tc.tile_pool(name="pvp", bufs=2, space="PSUM"))

        # ---- constants ----
        ebias = consts.tile([128, 2], F32, name="ebias")  # exp bias: [0]=0, [1]=ctx -2
        nc.vector.memset(ebias[:, 0:1], 0.0)
        nc.vector.memset(ebias[:, 1:2], -2.0)
        ones64 = consts.tile([128, 64], BF16, name="ones64")
        nc.vector.memset(ones64, 1.0)
        maskband = consts.tile([128, 256], BF16, name="maskband")
        bias_qk = consts.tile([128, 8], F32, name="bias_qk")
        bias_kc = consts.tile([128, 4], F32, name="bias_kc")
        bvb = consts.tile([128, 512], BF16, name="bvb")
        bvcb = consts.tile([128, 512], BF16, name="bvcb")

        # ---- SBUF activation/weight tiles ----
        ctx_sb = wts.tile([128, 8, Tc], BF16, name="ctx_sb")
        wkc_sb = wts.tile([128, 4, 8, 128], BF16, name="wkc_sb")
        wvc_sb = wts.tile([128, 8, 512], BF16, name="wvc_sb")
        wq_sb = wts.tile([128, 4, 8, 128], BF16, name="wq_sb")
        wk_sb = wts.tile([128, 4, 8, 128], BF16, name="wk_sb")
        wv_sb = wts.tile([128, 8, 512], BF16, name="wv_sb")
        wpj_sb = wts.tile([128, 4, 1024], BF16, name="wpj_sb")
        x0_sb = xp.tile([128, 8, 512], BF16, name="x0_sb")
        x1_sb = xp.tile([128, 8, 384], BF16, name="x1_sb")

        # ---- input loads ----
        # sync HWDGE queue: critical-path order
        nc.sync.dma_start(out=ctx_sb, in_=ctx_h[:, :, :])
        nc.sync.dma_start(out=wkc_sb[:, 0, :, :], in_=wkc_h[:, 0, :, :])
        nc.sync.dma_start(out=x0_sb[:, :, 0:256], in_=x0_h[:, :, 0:256])
        nc.sync.dma_start(out=wkc_sb[:, 1:4, :, :], in_=wkc_h[:, 1:4, :, :])
        nc.sync.dma_start(out=x0_sb[:, :, 256:512], in_=x0_h[:, :, 256:512])
        for f in range(4):
            nc.sync.dma_start(out=wq_sb[:, f, :, :], in_=wq_h[:, f, :, :])
            nc.sync.dma_start(out=wk_sb[:, f, :, :], in_=wk_h[:, f, :, :])
        nc.sync.dma_start(out=x1_sb, in_=x1_h[:, :, :])
        nc.sync.dma_start(out=wpj_sb, in_=wpj_h[:, :, :])
        # scalar HWDGE queue: consts first (unblock ctx-proj drains), then bulk v
        nc.scalar.dma_start(out=bias_kc, in_=bkc_h[:, :])
        nc.scalar.dma_start(out=bias_qk, in_=bqk_h[:, :])
        nc.scalar.dma_start(out=maskband, in_=mb_h[:, :])
        nc.scalar.dma_start(out=wv_sb[:, 0:4, :], in_=wv_h[:, 0:4, :])
        nc.scalar.dma_start(out=wv_sb[:, 4:8, :], in_=wv_h[:, 4:8, :])
        nc.scalar.dma_start(out=bvb, in_=bv_h[:, :])
        nc.scalar.dma_start(out=bvcb, in_=bvc_h[:, :])
        nc.scalar.dma_start(out=wvc_sb, in_=wvc_h[:, :, :])

        # ---- persistent activation tiles (token-chunk-split: clean deps) ----
        qT0 = [qkp.tile([128, 512], BF16, name=f"qT0_{p}") for p in range(NPAIR)]
        qT1 = [qkp.tile([128, 384], BF16, name=f"qT1_{p}") for p in range(NPAIR)]
        kTa = [qkp.tile([128, 640], BF16, name=f"kTa{p}") for p in range(NPAIR)]
        kTb = [qkp.tile([128, 384], BF16, name=f"kTb{p}") for p in range(NPAIR)]
        v_sb = [vtp.tile([128, 8, 64], BF16, name=f"v{c}") for c in range(8)]
        aT0 = [atp.tile([128, 512], BF16, name=f"aT0_{p}") for p in range(NPAIR)]
        aT1 = [atp.tile([128, 384], BF16, name=f"aT1_{p}") for p in range(NPAIR)]

        def kt_slice(p, c):
            """kT columns [128c, 128c+128) of pair p (ctx + k concatenated)."""
            if c <= 4:
                return kTa[p][:, 128 * c:128 * c + 128]
            return kTb[p][:, 128 * c - 640:128 * c - 512]

        def x_slice(kc, ts, te):
            if te <= 512:
                return x0_sb[:, kc, ts:te]
            return x1_sb[:, kc, ts - 512:te - 512]

        # ---- ctx projections: kcT -> kTa cols 0:128, vc -> v_sb[0] ----
        for f in range(4):
            pt = pps.tile([128, 512], F32, tag="pp", name=f"pkc{f}")
            for kc in range(8):
                nc.tensor.matmul(
                    pt[:, 0:Tc], wkc_sb[:, f, kc, :],
                    ctx_sb[:, kc, :], start=(kc == 0), stop=(kc == 7))
            nc.scalar.activation(
                out=kTa[f][:, 0:Tc], in_=pt[:, 0:Tc], func=AF.Identity,
                bias=bias_kc[:, f:f + 1], scale=1.0)
        # ---- v projection (natural layout) ----
        def v_proj(tt):
            pt = pps.tile([128, 512], F32, tag="pp", name=f"pv{tt}")
            for kc in range(8):
                nc.tensor.matmul(
                    pt[:, 0:512], x_slice(kc, 128 * tt, 128 * tt + 128),
                    wv_sb[:, kc, :], start=(kc == 0), stop=(kc == 7))
            nc.vector.tensor_add(
                out=v_sb[1 + tt][:, :, :],
                in0=pt[:, 0:512].rearrange("p (h d) -> p h d", h=8),
                in1=bvb.rearrange("p (h d) -> p h d", h=8))

        for tt in range(4):
            v_proj(tt)

        pt = pps.tile([128, 512], F32, tag="pp", name="pvc")
        for kc in range(8):
            nc.tensor.matmul(
                pt[:, 0:512], ctx_sb[:, kc, :], wvc_sb[:, kc, :],
                start=(kc == 0), stop=(kc == 7))
        nc.vector.tensor_add(
            out=v_sb[0][:, :, :],
            in0=pt[:, 0:512].rearrange("p (h d) -> p h d", h=8),
            in1=bvcb.rearrange("p (h d) -> p h d", h=8))

        # ---- qT / kT projections (transposed layout), per token chunk ----
        def qk_ftile(w_sb, f, dest, dcol, bias_col, ts, te, drain):
            pt = pps.tile([128, 512], F32, tag="pp", name=f"pqk{bias_col}{ts}")
            for kc in range(8):
                nc.tensor.matmul(
                    pt[:, 0:te - ts], w_sb[:, f, kc, :],
                    x_slice(kc, ts, te), start=(kc == 0), stop=(kc == 7))
            if drain == "act":
                nc.scalar.activation(
                    out=dest[:, dcol:dcol + te - ts], in_=pt[:, 0:te - ts],
                    func=AF.Identity, bias=bias_qk[:, bias_col:bias_col + 1],
                    scale=1.0)
            else:
                nc.vector.tensor_scalar_add(
                    out=dest[:, dcol:dcol + te - ts], in0=pt[:, 0:te - ts],
                    scalar1=bias_qk[:, bias_col:bias_col + 1])

        def attn(p, t_i):
            ts, te = TCH[t_i]
            n = te - ts
            last_c = 4 if t_i == 0 else 7
            qT = qT0[p] if t_i == 0 else qT1[p]
            aT = aT0[p] if t_i == 0 else aT1[p]
            pa = pvp.tile([128, 512], F32, tag="pa", name=f"pa{p}{t_i}")
            dsum = dsp.tile([128, 2, 512], BF16, tag="ds", name=f"ds{p}{t_i}")
            chunks = [c for c in range(8) if _rect(c, ts, te) is not None]

            def scores(c):
                cs, _ = _rect(c, ts, te)
                sc = scp.tile([128, 2, 512], F32, tag="sc", name=f"sc{p}{t_i}{c}")
                for hi in range(2):
                    nc.tensor.matmul(
                        sc[:, hi, cs - ts:n],
                        kt_slice(p, c)[64 * hi:64 * hi + 64, :],
                        qT[64 * hi:64 * hi + 64, cs - ts:n],
                        start=True, stop=True, tile_position=(64 * hi, 0))
                pb = probsp.tile([128, 2, 512], BF16, tag="pb", name=f"pb{p}{t_i}{c}")
                nc.scalar.activation(
                    out=pb[:, :, cs - ts:n], in_=sc[:, :, cs - ts:n],
                    func=AF.Exp,
                    bias=(ebias[:, 1:2] if c == 0 else ebias[:, 0:1]),
                    scale=0.125)
                for hi in range(2):
                    for s0, e0, mc in _band_pieces(c, ts, te):
                        nc.vector.tensor_mul(
                            out=pb[:, hi, s0 - ts:e0 - ts],
                            in0=pb[:, hi, s0 - ts:e0 - ts],
                            in1=maskband[:, mc:mc + (e0 - s0)])
                return pb

            def pv(c, pb):
                cs, _ = _rect(c, ts, te)
                for hi in range(2):
                    nc.tensor.matmul(
                        pa[64 * hi:64 * hi + 64, cs - ts:n],
                        v_sb[c][:, 2 * p + hi, :],
                        pb[:, hi, cs - ts:n],
                        start=(c == 0), stop=(c == last_c),
                        skip_group_check=True, tile_position=(0, 64 * hi))
                if c == 0:
                    nc.vector.tensor_copy(out=dsum[:, :, 0:n], in_=pb[:, :, 0:n])
                else:
                    nc.vector.tensor_add(
                        out=dsum[:, :, cs - ts:n], in0=dsum[:, :, cs - ts:n],
                        in1=pb[:, :, cs - ts:n])

            pending = None
            for c in chunks:
                pb = scores(c)
                if pending is not None:
                    pv(*pending)
                pending = (c, pb)
            pv(*pending)
            pd = scp.tile([128, 2, 512], F32, tag="sc", name=f"pd{p}{t_i}")
            for hi in range(2):
                nc.tensor.matmul(
                    pd[64 * hi:64 * hi + 64, 0, 0:n], ones64, dsum[:, hi, 0:n],
                    start=True, stop=True, tile_position=(0, 64 * hi),
                    skip_group_check=True)
            rb = rbp.tile([128, 512], F32, tag="rb", name=f"rb{p}{t_i}")
            nc.vector.reciprocal(out=rb[:, 0:n], in_=pd[:, 0, 0:n])
            nc.vector.tensor_mul(out=aT[:, 0:n], in0=pa[:, 0:n], in1=rb[:, 0:n])

        def outproj(t_i, ofs):
            ts, te = TCH[t_i]
            n = te - ts
            aT = aT0 if t_i == 0 else aT1
            for of in ofs:
                pt = pps.tile([128, 512], F32, tag="pp", name=f"po{of}{t_i}")
                for kc in range(4):
                    nc.tensor.matmul(
                        pt[:, 0:n], wpj_sb[:, kc, 128 * of:128 * of + 128],
                        aT[kc][:, 0:n], start=(kc == 0), stop=(kc == 3))
                # t0 drains on DVE (ACT busy with t1 exps); t1 drains on ACT
                # (free after the last exp, DVE busy with denominators).
                # out DMAs alternate between the two HWDGE queues.
                ob = outp.tile([128, 512], BF16, tag="ob", name=f"ob{of}{t_i}")
                if t_i == 0 or of == 7:
                    nc.vector.tensor_copy(out=ob[:, 0:n], in_=pt[:, 0:n])
                else:
                    nc.scalar.copy(out=ob[:, 0:n], in_=pt[:, 0:n])
                eng = nc.sync if of % 2 == 0 else nc.scalar
                if t_i == 1 and of == 7:
                    eng = nc.sync
                eng.dma_start(out=out_h[128 * of:128 * of + 128, ts:te], in_=ob[:, 0:n])

        def outproj_final(t_i):
            # of-pairs on scp-pool tiles (scores are done; 4 of-tiles in flight)
            ts, te = TCH[t_i]
            n = te - ts
            aT = aT0 if t_i == 0 else aT1
            for og in range(4):
                pt = scp.tile([128, 2, 512], F32, tag="sc", name=f"pg{og}{t_i}")
                for i in range(2):
                    of = 2 * og + i
                    for kc in range(4):
                        nc.tensor.matmul(
                            pt[:, i, 0:n], wpj_sb[:, kc, 128 * of:128 * of + 128],
                            aT[kc][:, 0:n], start=(kc == 0), stop=(kc == 3))
                ob = outp.tile([128, 2, 512], BF16, tag="obg", name=f"obg{og}{t_i}")
                nc.vector.tensor_copy(out=ob[0:64, :, 0:n], in_=pt[0:64, :, 0:n])
                nc.scalar.copy(out=ob[64:128, :, 0:n], in_=pt[64:128, :, 0:n])
                for i in range(2):
                    of = 2 * og + i
                    nc.sync.dma_start(
                        out=out_h[128 * of:128 * of + 64, ts:te], in_=ob[0:64, i, 0:n])
                    nc.scalar.dma_start(
                        out=out_h[128 * of + 64:128 * of + 128, ts:te],
                        in_=ob[64:128, i, 0:n])

        # ---- schedule: pair-major pipeline; outproj fills trailing stalls ----
        for p in range(NPAIR):
            qk_ftile(wq_sb, p, qT0[p], 0, p, 0, 512, "act")
            qk_ftile(wk_sb, p, kTa[p], Tc, 4 + p, 0, 512, "dve")
            attn(p, 0)
        for tt in range(4, 7):
            v_proj(tt)
        for p in range(NPAIR):
            qk_ftile(wq_sb, p, qT1[p], 0, p, 512, 896, "act")
            qk_ftile(wk_sb, p, kTb[p], 0, 4 + p, 512, 896, "dve")
            attn(p, 1)
            outproj(0, range(2 * p, 2 * p + 2))
        outproj(1, range(8))

    if not nc.is_finalized():
        nc.finalize()
    return nc


_NC_CACHE = {}


def _get_nc():
    if "nc" not in _NC_CACHE:
        _NC_CACHE["nc"] = build_nc()
    return _NC_CACHE["nc"]


def _pack128(v):
    """[128*n] -> [128, n] with [p, f] = v[128*f + p]."""
    n = v.shape[0] // 128
    return np.ascontiguousarray(v.reshape(n, 128).T)


def _kc_major(w):
    """[1024, F] -> [128, 8, F]: partition-major with kc chunks."""
    F = w.shape[1]
    return np.ascontiguousarray(w.reshape(8, 128, F).transpose(1, 0, 2))


def _f_major(w):
    """[1024, 512] -> [128, 4 fblk, 8 kc, 128]."""
    r = w.reshape(8, 128, 4, 128)  # kc, p, f, ff
    return np.ascontiguousarray(r.transpose(1, 2, 0, 3))


def make_in_maps(inputs):
    bf16 = ml_dtypes.bfloat16
    x = np.asarray(inputs["x"], np.float32)
    ctx_seq = np.asarray(inputs["context_seq"], np.float32)
    w_ref = np.asarray(inputs["w_ref"], np.float32)
    b_ref = np.asarray(inputs["b_ref"], np.float32)
    w_attn = np.asarray(inputs["w_attn"], np.float32)
    b_attn = np.asarray(inputs["b_attn"], np.float32)
    w_proj = np.asarray(inputs["w_proj"], np.float32)

    # mask band constant: cols 0-127 causal (1 where q>=p), cols 128-255
    # anti-diagonal (0 where q==p else 1)
    qq = np.arange(128)[None, :]
    pp = np.arange(128)[:, None]
    mband = np.ascontiguousarray(
        np.concatenate([(qq >= pp), (qq != pp)], axis=1).astype(bf16))

    in_maps = []
    for b in range(4):
        xT = x[b].T.astype(bf16)  # [1024, 896]
        x0r = _kc_major(xT[:, 0:512])
        x1r = _kc_major(xT[:, 512:896])
        ctxr = _kc_major(ctx_seq[b].T.astype(bf16))
        for g in range(2):
            sl = slice(512 * g, 512 * g + 512)
            in_maps.append(dict(
                x0r=x0r,
                x1r=x1r,
                ctxr=ctxr,
                w_q=_f_major(w_attn[:, 0 * NX:1 * NX][:, sl].astype(bf16)),
                w_k=_f_major(w_attn[:, 1 * NX:2 * NX][:, sl].astype(bf16)),
                w_v=_kc_major(w_attn[:, 2 * NX:3 * NX][:, sl].astype(bf16)),
                w_kc=_f_major(w_ref[:, 0 * NX:1 * NX][:, sl].astype(bf16)),
                w_vc=_kc_major(w_ref[:, 1 * NX:2 * NX][:, sl].astype(bf16)),
                w_pj=np.ascontiguousarray(
                    w_proj[sl, :].astype(bf16).reshape(4, 128, NX).transpose(1, 0, 2)),
                b_qk=_pack128(np.concatenate([b_attn[0 * NX:1 * NX][sl],
                                              b_attn[1 * NX:2 * NX][sl]])),
                b_kc=_pack128(b_ref[0 * NX:1 * NX][sl]),
                b_v=np.ascontiguousarray(np.broadcast_to(
                    b_attn[2 * NX:3 * NX][sl].astype(bf16), (128, 512))),
                b_vc=np.ascontiguousarray(np.broadcast_to(
                    b_ref[1 * NX:2 * NX][sl].astype(bf16), (128, 512))),
                mband=mband,
            ))
    return in_maps


def kernel(**inputs):
    b_proj = np.asarray(inputs["b_proj"], np.float32)
    in_maps = make_in_maps(inputs)
    nc = _get_nc()
    res = run_bass_kernel_spmd(nc, in_maps, core_ids=list(range(8)),
                               trace=os.environ.get("COCON_TRACE", "") == "1")
    outs = res.results
    out = np.empty((4, T, NX), np.float32)
    for b in range(4):
        acc = (outs[2 * b]["outT"].astype(np.float32)
               + outs[2 * b + 1]["outT"].astype(np.float32))  # [1024, 896]
        out[b] = acc.T + b_proj[None, :]
    if res.exec_time_ns is not None:
        kernel.last_exec_time_ns = res.exec_time_ns
    return out


kernel.last_exec_time_ns = None
